# revision 9
# baseline (speedup 1.0000x reference)
"""Trainium2 Bass kernel for BioBERT-ARG-GNN (gated pooling + 2-layer GCN + MLP head).

Strategy: pure data parallel over batch B=64 across 8 NeuronCores (8 graphs
per core).  All segment/gather ops are expressed as dense matmuls against
one-hot matrices built on-device from the index tensors (N=128 nodes ==
partition dim).  GCN normalization (D^-1/2 (A+I) D^-1/2) factors into
per-partition scalar scalings around a dense [128,128] adjacency matmul.
bf16 is used on the matmul paths (exact for one-hot/adjacency counts) since
fp32 matmuls run as two half-speed passes on the PE.  Phase split: all
graphs' adjacency/degree work first (one ACT Sqrt table load), then the
gated pooling + GCN per graph (one Sigmoid table load).
"""

import os
import sys

import numpy as np

for _p in ("/opt/trn_rl_repo", "/root/.axon_site/_ro/trn_rl_repo"):
    if os.path.isdir(_p) and _p not in sys.path:
        sys.path.insert(0, _p)

import ml_dtypes  # noqa: E402
import concourse.bass as bass  # noqa: E402
import concourse.mybir as mybir  # noqa: E402
from concourse import tile  # noqa: E402
from concourse.bass_utils import run_bass_kernel_spmd  # noqa: E402

# Problem shapes (hardcoded per contest rules).
B, S, H = 64, 512, 768
N, E = 128, 1024
GH, FH, L = 128, 256, 2
NCORES = 8
BL = B // NCORES  # graphs per core
SC = S // 128     # subtoken chunks per graph
EC = E // 128     # edge chunks per graph
HC = H // 128     # BERT-hidden chunks
FC = (H + GH) // 128  # concat-feature chunks for the FC head

f32 = mybir.dt.float32
bf16 = mybir.dt.bfloat16
AFT = mybir.ActivationFunctionType
ALU = mybir.AluOpType
BF16 = ml_dtypes.bfloat16

_CACHE = {}


def _split_multi_waits(nc: bass.Bass) -> int:
    """Walrus in this container accepts one sync-wait per instruction; split
    extra waits into single-wait EventSemaphore nops just before it."""
    n_split = 0
    for fn in nc.m.functions:
        for blk in fn.blocks:
            new_instrs = []
            changed = False
            for inst in blk.instructions:
                si = getattr(inst, "sync_info", None)
                if si is not None and si.on_wait is not None and len(si.on_wait) > 1:
                    waits = list(si.on_wait)
                    for j, w in enumerate(waits[:-1]):
                        ev = mybir.InstEventSemaphore(
                            name=f"{inst.name}_ws{j}",
                            ins=[], outs=[],
                            engine=inst.engine,
                            sync_info=mybir.SyncInfo(on_wait=[w], on_update=[]),
                        )
                        new_instrs.append(ev)
                    inst.sync_info = mybir.SyncInfo(
                        on_wait=[waits[-1]], on_update=list(si.on_update))
                    n_split += 1
                    changed = True
                new_instrs.append(inst)
            if changed:
                blk.instructions = new_instrs
    return n_split


def build_program(br_val: float) -> bass.Bass:
    nc = bass.Bass()

    lh_d = nc.declare_dram_parameter("lh", [BL, S, H], f32, isOutput=False)
    subv_d = nc.declare_dram_parameter("subv", [BL, 128, SC], f32, isOutput=False)
    esrc_d = nc.declare_dram_parameter("esrc", [BL, 128, EC], f32, isOutput=False)
    edst_d = nc.declare_dram_parameter("edst", [BL, 128, EC], f32, isOutput=False)
    wrb_d = nc.declare_dram_parameter("wrb", [128, H], bf16, isOutput=False)
    w1t_d = nc.declare_dram_parameter("w1t", [128, HC, GH], bf16, isOutput=False)
    w2t_d = nc.declare_dram_parameter("w2t", [GH, GH], bf16, isOutput=False)
    wf1t_d = nc.declare_dram_parameter("wf1t", [128, FC, FH], f32, isOutput=False)
    wf2t_d = nc.declare_dram_parameter("wf2t", [128, 2, L], f32, isOutput=False)
    b1b_d = nc.declare_dram_parameter("b1b", [128, GH], f32, isOutput=False)
    b2b_d = nc.declare_dram_parameter("b2b", [128, GH], f32, isOutput=False)
    bf1b_d = nc.declare_dram_parameter("bf1b", [BL, FH], f32, isOutput=False)
    bf2b_d = nc.declare_dram_parameter("bf2b", [BL, L], f32, isOutput=False)
    iotab_d = nc.declare_dram_parameter("iota_b", [128, 128], bf16, isOutput=False)
    identb_d = nc.declare_dram_parameter("ident_b", [128, 128], bf16, isOutput=False)
    identf_d = nc.declare_dram_parameter("ident_f", [128, 128], f32, isOutput=False)
    onesb_d = nc.declare_dram_parameter("ones_b", [128, 1], bf16, isOutput=False)
    meanb_d = nc.declare_dram_parameter("mean_b", [128, 1], bf16, isOutput=False)
    out_d = nc.declare_dram_parameter("out", [BL, L], f32, isOutput=True)

    with tile.TileContext(nc) as tc:
        with (
            tc.tile_pool(name="const", bufs=1) as cpool,
            tc.tile_pool(name="lhp", bufs=6) as lhpool,
            tc.tile_pool(name="scr", bufs=2) as scpool,
            tc.tile_pool(name="work", bufs=3) as wpool,
            tc.tile_pool(name="small", bufs=6) as spool,
            tc.tile_pool(name="psA", bufs=2, space="PSUM") as psA,
            tc.tile_pool(name="psB", bufs=2, space="PSUM") as psB,
            tc.tile_pool(name="psC", bufs=1, space="PSUM") as psC,
        ):
            # ---- constants (loaded once) ----
            iota_b = cpool.tile([128, 128], bf16)
            nc.sync.dma_start(iota_b[:], iotab_d[:])
            ident_b = cpool.tile([128, 128], bf16)
            nc.sync.dma_start(ident_b[:], identb_d[:])
            ident_f = cpool.tile([128, 128], f32)
            nc.sync.dma_start(ident_f[:], identf_d[:])
            wrb = cpool.tile([128, H], bf16)
            nc.sync.dma_start(wrb[:], wrb_d[:])
            w1s = cpool.tile([128, HC, GH], bf16)
            nc.sync.dma_start(w1s[:], w1t_d[:])
            w2s = cpool.tile([GH, GH], bf16)
            nc.sync.dma_start(w2s[:], w2t_d[:])
            wf1s = cpool.tile([128, FC, FH], f32)
            nc.sync.dma_start(wf1s[:], wf1t_d[:])
            wf2s = cpool.tile([128, 2, L], f32)
            nc.sync.dma_start(wf2s[:], wf2t_d[:])
            b1b = cpool.tile([128, GH], f32)
            nc.sync.dma_start(b1b[:], b1b_d[:])
            b2b = cpool.tile([128, GH], f32)
            nc.sync.dma_start(b2b[:], b2b_d[:])
            bf1b = cpool.tile([BL, FH], f32)
            nc.sync.dma_start(bf1b[:], bf1b_d[:])
            bf2b = cpool.tile([BL, L], f32)
            nc.sync.dma_start(bf2b[:], bf2b_d[:])
            ones_b = cpool.tile([128, 1], bf16)
            nc.sync.dma_start(ones_b[:], onesb_d[:])
            mean_b = cpool.tile([128, 1], bf16)
            nc.sync.dma_start(mean_b[:], meanb_d[:])
            # pooled graph embeddings (written one column per graph)
            catT6 = cpool.tile([128, BL], f32)

            # ---------- phase 0: adjacency + degrees for all graphs ----------
            atis = []
            dinvs = []
            subvs = []
            for g in range(BL):
                subv = spool.tile([128, SC], f32, tag="subv", bufs=BL)
                nc.sync.dma_start(subv[:], subv_d[g])
                subvs.append(subv)
                esrc = spool.tile([128, EC], f32, tag="esrc", bufs=2)
                nc.sync.dma_start(esrc[:], esrc_d[g])
                edst = spool.tile([128, EC], f32, tag="edst", bufs=2)
                nc.sync.dma_start(edst[:], edst_d[g])

                at_ps = psB.tile([128, 128], f32, tag="mm")
                for c in range(EC):
                    s_t = wpool.tile([128, 128], bf16, tag="ohS")
                    d_t = wpool.tile([128, 128], bf16, tag="ohD")
                    nc.gpsimd.tensor_scalar(s_t[:], iota_b[:], esrc[:, c : c + 1],
                                            None, ALU.is_equal)
                    nc.vector.tensor_scalar(d_t[:], iota_b[:], edst[:, c : c + 1],
                                            None, ALU.is_equal)
                    nc.tensor.matmul(at_ps[:], s_t[:], d_t[:], start=(c == 0),
                                     stop=(c == EC - 1))
                # ATI = AT + I  (adds the self-loops); exact in bf16
                ati = wpool.tile([128, 128], bf16, tag="ati", bufs=BL)
                nc.vector.scalar_tensor_tensor(ati[:], at_ps[:], 1.0, ident_b[:],
                                               ALU.bypass, ALU.add)
                atis.append(ati)
                # deg[d] = sum_s ATI[s,d]  -> dinv = 1/sqrt(deg)
                deg_ps = psB.tile([128, 1], f32, tag="mm")
                nc.tensor.matmul(deg_ps[:], ati[:], ones_b[:], start=True, stop=True)
                sdeg = spool.tile([128, 1], f32, tag="sv")
                nc.scalar.activation(sdeg[:], deg_ps[:], AFT.Sqrt)
                dinv = spool.tile([128, 1], f32, tag="dinv", bufs=BL)
                nc.vector.reciprocal(dinv[:], sdeg[:])
                dinvs.append(dinv)

            # ---------- phase 1: gate + pooling + GCN per graph ----------
            for g in range(BL):
                subv = subvs[g]
                ati = atis[g]
                dinv = dinvs[g]

                lhbs = []
                pgs = []
                cnt_ps = psC.tile([128, SC], f32, tag="v")
                for c in range(SC):
                    lht = lhpool.tile([128, H], f32, tag="lh")
                    nc.sync.dma_start(lht[:], lh_d[g, c * 128 : (c + 1) * 128, :])
                    lhb = lhpool.tile([128, H], bf16, tag="lhb", bufs=6)
                    nc.gpsimd.tensor_copy(lhb[:], lht[:])
                    scr = scpool.tile([128, H], bf16, tag="scr")
                    logits = spool.tile([128, 1], f32, tag="sv")
                    nc.vector.scalar_tensor_tensor(
                        scr[:], lhb[:], 0.0, wrb[:], ALU.bypass, ALU.mult,
                        accum_out=logits[:])
                    gate = spool.tile([128, 1], f32, tag="sv")
                    nc.scalar.activation(gate[:], logits[:], AFT.Sigmoid,
                                         bias=float(br_val))
                    p_t = wpool.tile([128, 128], bf16, tag="ohP")
                    pg_t = wpool.tile([128, 128], bf16, tag="ohPg", bufs=6)
                    nc.vector.tensor_scalar(p_t[:], iota_b[:], subv[:, c : c + 1],
                                            None, ALU.is_equal)
                    nc.vector.tensor_scalar(pg_t[:], p_t[:], gate[:], None, ALU.mult)
                    nc.tensor.matmul(cnt_ps[:, c : c + 1], p_t[:], ones_b[:],
                                     start=True, stop=True)
                    lhbs.append(lhb)
                    pgs.append(pg_t)

                # pooled node feats (transposed): nfT[h, n] = sum_s lh[s,h]*Pg[s,n]
                nf_ps = psA.tile([128, HC, GH], f32, tag="nf")
                for hc in range(HC):
                    for c in range(SC):
                        nc.tensor.matmul(nf_ps[:, hc, :],
                                         lhbs[c][:, hc * 128 : (hc + 1) * 128],
                                         pgs[c][:],
                                         start=(c == 0), stop=(c == SC - 1))

                # 1/max(cnt,1); combined layer-1 row scale s1 = invc * dinv
                cnt1 = spool.tile([128, 1], f32, tag="sv")
                nc.vector.tensor_reduce(cnt1[:], cnt_ps[:], mybir.AxisListType.X,
                                        ALU.add)
                mx = spool.tile([128, 1], f32, tag="sv")
                nc.vector.tensor_scalar_max(mx[:], cnt1[:], 1.0)
                invc = spool.tile([128, 1], f32, tag="sv")
                nc.vector.reciprocal(invc[:], mx[:])
                s1 = spool.tile([128, 1], f32, tag="sv")
                nc.vector.tensor_tensor(s1[:], invc[:], dinv[:], ALU.mult)

                nfs = wpool.tile([128, HC, GH], bf16, tag="nfs", bufs=2)
                for hc in range(HC):
                    nc.any.tensor_copy(nfs[:, hc, :], nf_ps[:, hc, :])

                # GCN layer 1
                t1_ps = psB.tile([128, GH], f32, tag="mm")
                for hc in range(HC):
                    nc.tensor.matmul(t1_ps[:], nfs[:, hc, :], w1s[:, hc, :],
                                     start=(hc == 0), stop=(hc == HC - 1))
                t2 = wpool.tile([128, GH], bf16, tag="t2")
                nc.vector.tensor_scalar_mul(t2[:], t1_ps[:], s1[:])
                z_ps = psB.tile([128, GH], f32, tag="mm")
                nc.tensor.matmul(z_ps[:], ati[:], t2[:], start=True, stop=True)
                x1p = wpool.tile([128, GH], f32, tag="x1p")
                nc.vector.scalar_tensor_tensor(x1p[:], z_ps[:], dinv[:], b1b[:],
                                               ALU.mult, ALU.add)
                x1 = wpool.tile([128, GH], bf16, tag="x1")
                nc.gpsimd.tensor_scalar_max(x1[:], x1p[:], 0.0)

                # GCN layer 2
                x1t_ps = psB.tile([128, GH], bf16, tag="mmb", bufs=1)
                nc.tensor.transpose(x1t_ps[:], x1[:], ident_b[:])
                x1t = wpool.tile([128, GH], bf16, tag="x1t")
                nc.any.tensor_copy(x1t[:], x1t_ps[:])
                tp_ps = psB.tile([128, GH], f32, tag="mm")
                nc.tensor.matmul(tp_ps[:], x1t[:], w2s[:], start=True, stop=True)
                t2p = wpool.tile([128, GH], bf16, tag="t2")
                nc.vector.tensor_scalar_mul(t2p[:], tp_ps[:], dinv[:])
                z2_ps = psB.tile([128, GH], f32, tag="mm")
                nc.tensor.matmul(z2_ps[:], ati[:], t2p[:], start=True, stop=True)
                x2p = wpool.tile([128, GH], f32, tag="x1p")
                nc.vector.scalar_tensor_tensor(x2p[:], z2_ps[:], dinv[:], b2b[:],
                                               ALU.mult, ALU.add)
                x2 = wpool.tile([128, GH], bf16, tag="x1")
                nc.gpsimd.tensor_scalar_max(x2[:], x2p[:], 0.0)

                # graph mean pool -> column g of catT6
                pool_ps = psC.tile([128, 1], f32, tag="v")
                nc.tensor.matmul(pool_ps[:], x2[:], mean_b[:], start=True,
                                 stop=True)
                nc.any.tensor_copy(catT6[:, g : g + 1], pool_ps[:])

            # ---------- FC head over all BL graphs ----------
            clsr = cpool.tile([BL, H], f32)
            nc.sync.dma_start(clsr[:], lh_d[:, 0, :])
            h1_ps = psB.tile([BL, FH], f32, tag="mm")
            for c in range(FC):
                if c < HC:
                    ct_ps = psB.tile([128, BL], f32, tag="mm")
                    nc.tensor.transpose(ct_ps[:], clsr[:, c * 128 : (c + 1) * 128],
                                        ident_f[0:BL, 0:BL])
                    catc = wpool.tile([128, BL], f32, tag="catc", bufs=2)
                    nc.any.tensor_copy(catc[:], ct_ps[:])
                else:
                    catc = catT6
                nc.tensor.matmul(h1_ps[:], catc[:], wf1s[:, c, :], start=(c == 0),
                                 stop=(c == FC - 1))
            h1s = wpool.tile([BL, FH], f32, tag="h1")
            nc.vector.scalar_tensor_tensor(h1s[:], h1_ps[:], 1.0, bf1b[:],
                                           ALU.bypass, ALU.add)
            hr = wpool.tile([BL, FH], f32, tag="h1")
            nc.scalar.activation(hr[:], h1s[:], AFT.Relu)
            out_ps = psC.tile([BL, L], f32, tag="v")
            for c in range(2):
                ht_ps = psB.tile([128, BL], f32, tag="mm")
                nc.tensor.transpose(ht_ps[:], hr[:, c * 128 : (c + 1) * 128],
                                    ident_f[0:BL, 0:BL])
                htc = wpool.tile([128, BL], f32, tag="catc", bufs=2)
                nc.any.tensor_copy(htc[:], ht_ps[:])
                nc.tensor.matmul(out_ps[:], htc[:], wf2s[:, c, :], start=(c == 0),
                                 stop=(c == 1))
            outs = wpool.tile([BL, L], f32, tag="outs")
            nc.vector.scalar_tensor_tensor(outs[:], out_ps[:], 1.0, bf2b[:],
                                           ALU.bypass, ALU.add)
            nc.sync.dma_start(out_d[:], outs[:])

    _split_multi_waits(nc)
    return nc


def _prepare_in_maps(inputs):
    lh = np.ascontiguousarray(np.asarray(inputs["last_hidden"], dtype=np.float32))
    submap = np.asarray(inputs["submap"]).astype(np.int64)
    edge_index = np.asarray(inputs["edge_index"]).astype(np.int64)
    assert lh.shape == (B, S, H)
    assert int(inputs.get("num_nodes", N)) == N

    wr = np.asarray(inputs["wr"], dtype=np.float32)
    br = float(np.asarray(inputs["br"], dtype=np.float32))
    W1 = np.asarray(inputs["W1"], dtype=np.float32)
    b1 = np.asarray(inputs["b1"], dtype=np.float32)
    W2 = np.asarray(inputs["W2"], dtype=np.float32)
    b2 = np.asarray(inputs["b2"], dtype=np.float32)
    Wf1 = np.asarray(inputs["Wf1"], dtype=np.float32)
    bf1 = np.asarray(inputs["bf1"], dtype=np.float32)
    Wf2 = np.asarray(inputs["Wf2"], dtype=np.float32)
    bf2 = np.asarray(inputs["bf2"], dtype=np.float32)

    # Shared (replicated) tensors.
    consts = {
        "wrb": np.ascontiguousarray(np.broadcast_to(wr, (128, H))).astype(BF16),
        "w1t": np.ascontiguousarray(
            W1.reshape(HC, 128, GH).transpose(1, 0, 2)).astype(BF16),
        "w2t": np.ascontiguousarray(W2).astype(BF16),
        "wf1t": np.ascontiguousarray(
            Wf1.reshape(FC, 128, FH).transpose(1, 0, 2)),
        "wf2t": np.ascontiguousarray(
            Wf2.reshape(2, 128, L).transpose(1, 0, 2)),
        "b1b": np.ascontiguousarray(np.broadcast_to(b1, (128, GH))),
        "b2b": np.ascontiguousarray(np.broadcast_to(b2, (128, GH))),
        "bf1b": np.ascontiguousarray(np.broadcast_to(bf1, (BL, FH))),
        "bf2b": np.ascontiguousarray(np.broadcast_to(bf2, (BL, L))),
        "iota_b": np.ascontiguousarray(
            np.broadcast_to(np.arange(128, dtype=np.float32).astype(BF16),
                            (128, 128))),
        "ident_b": np.eye(128, dtype=np.float32).astype(BF16),
        "ident_f": np.eye(128, dtype=np.float32),
        "ones_b": np.ones((128, 1), np.float32).astype(BF16),
        "mean_b": np.full((128, 1), 1.0 / N, np.float32).astype(BF16),
    }

    # Per-graph index layouts: value of token t goes to partition t%128,
    # column t//128.
    subv = submap.reshape(B, SC, 128).transpose(0, 2, 1).astype(np.float32)
    esrc = edge_index[:, 0, :].reshape(B, EC, 128).transpose(0, 2, 1).astype(np.float32)
    edst = edge_index[:, 1, :].reshape(B, EC, 128).transpose(0, 2, 1).astype(np.float32)

    in_maps = []
    for i in range(NCORES):
        sl = slice(i * BL, (i + 1) * BL)
        m = dict(consts)
        m["lh"] = np.ascontiguousarray(lh[sl])
        m["subv"] = np.ascontiguousarray(subv[sl])
        m["esrc"] = np.ascontiguousarray(esrc[sl])
        m["edst"] = np.ascontiguousarray(edst[sl])
        in_maps.append(m)
    return in_maps, br


def _run(inputs, trace=False):
    in_maps, br = _prepare_in_maps(inputs)
    key = ("prog", br)
    if key not in _CACHE:
        _CACHE[key] = build_program(br)
    nc = _CACHE[key]
    res = run_bass_kernel_spmd(nc, in_maps, list(range(NCORES)), trace=trace)
    out = np.concatenate([np.asarray(res.results[i]["out"]) for i in range(NCORES)],
                         axis=0).astype(np.float32)
    return out, res


def kernel(**inputs) -> np.ndarray:
    out, _ = _run(inputs, trace=False)
    return out


# revision 12
# speedup vs baseline: 2.2274x; 2.2274x over previous
"""Trainium2 Bass kernel for BioBERT-ARG-GNN (gated pooling + 2-layer GCN + MLP head).

Strategy: pure data parallel over batch B=64 across 8 NeuronCores (8 graphs
per core).  All segment/gather ops are dense matmuls against one-hot
matrices built on-device from the index tensors (N=128 nodes == partition
dim).  GCN normalization (D^-1/2 (A+I) D^-1/2) factors into per-partition
scalings around a dense [128,128] adjacency matmul.  Matmul dtypes: f32r
(TF32-like, 1 cycle/row at free-dim>=256) for the big subtoken pooling,
bf16 for the [128,128] GCN matmuls (adjacency counts are exact), f32 for
the tiny FC head.  Phase split keeps each ACT function's table loaded once.
"""

import os
import sys

import numpy as np

for _p in ("/opt/trn_rl_repo", "/root/.axon_site/_ro/trn_rl_repo"):
    if os.path.isdir(_p) and _p not in sys.path:
        sys.path.insert(0, _p)

import ml_dtypes  # noqa: E402
import concourse.bass as bass  # noqa: E402
import concourse.mybir as mybir  # noqa: E402
from concourse import tile  # noqa: E402
from concourse.bass_utils import run_bass_kernel_spmd  # noqa: E402

# Problem shapes (hardcoded per contest rules).
B, S, H = 64, 512, 768
N, E = 128, 1024
GH, FH, L = 128, 256, 2
NCORES = 8
BL = B // NCORES  # graphs per core
SC = S // 128     # subtoken chunks per graph
EC = E // 128     # edge chunks per graph
HC = H // 128     # BERT-hidden chunks
FC = (H + GH) // 128  # concat-feature chunks for the FC head

f32 = mybir.dt.float32
f32r = mybir.dt.float32r
bf16 = mybir.dt.bfloat16
AFT = mybir.ActivationFunctionType
ALU = mybir.AluOpType
BF16 = ml_dtypes.bfloat16

_CACHE = {}


def _split_multi_waits(nc: bass.Bass) -> int:
    """Walrus in this container accepts one sync-wait per instruction; split
    extra waits into single-wait EventSemaphore nops just before it."""
    n_split = 0
    for fn in nc.m.functions:
        for blk in fn.blocks:
            new_instrs = []
            changed = False
            for inst in blk.instructions:
                si = getattr(inst, "sync_info", None)
                if si is not None and si.on_wait is not None and len(si.on_wait) > 1:
                    waits = list(si.on_wait)
                    for j, w in enumerate(waits[:-1]):
                        ev = mybir.InstEventSemaphore(
                            name=f"{inst.name}_ws{j}",
                            ins=[], outs=[],
                            engine=inst.engine,
                            sync_info=mybir.SyncInfo(on_wait=[w], on_update=[]),
                        )
                        new_instrs.append(ev)
                    inst.sync_info = mybir.SyncInfo(
                        on_wait=[waits[-1]], on_update=list(si.on_update))
                    n_split += 1
                    changed = True
                new_instrs.append(inst)
            if changed:
                blk.instructions = new_instrs
    return n_split


def build_program(br_val: float, b1_zero: bool, b2_zero: bool) -> bass.Bass:
    nc = bass.Bass()

    lh_d = nc.declare_dram_parameter("lh", [BL, S, H], f32r, isOutput=False)
    subv_d = nc.declare_dram_parameter("subv", [BL, 128, SC], f32, isOutput=False)
    esrc_d = nc.declare_dram_parameter("esrc", [BL, 128, EC], f32, isOutput=False)
    edst_d = nc.declare_dram_parameter("edst", [BL, 128, EC], f32, isOutput=False)
    wrb_d = nc.declare_dram_parameter("wrb", [128, H], f32, isOutput=False)
    w1t_d = nc.declare_dram_parameter("w1t", [128, HC, GH], bf16, isOutput=False)
    w2t_d = nc.declare_dram_parameter("w2t", [GH, GH], bf16, isOutput=False)
    wf1t_d = nc.declare_dram_parameter("wf1t", [128, FC, FH], f32, isOutput=False)
    wf2t_d = nc.declare_dram_parameter("wf2t", [128, 2, L], f32, isOutput=False)
    b1b_d = nc.declare_dram_parameter("b1b", [128, GH], f32, isOutput=False)
    b2b_d = nc.declare_dram_parameter("b2b", [128, GH], f32, isOutput=False)
    bf1b_d = nc.declare_dram_parameter("bf1b", [BL, FH], f32, isOutput=False)
    bf2b_d = nc.declare_dram_parameter("bf2b", [BL, L], f32, isOutput=False)
    iotaf_d = nc.declare_dram_parameter("iota_f", [128, 128], f32, isOutput=False)
    identb_d = nc.declare_dram_parameter("ident_b", [128, 128], bf16, isOutput=False)
    identf_d = nc.declare_dram_parameter("ident_f", [128, 128], f32, isOutput=False)
    onesr_d = nc.declare_dram_parameter("ones_r", [128, 1], f32r, isOutput=False)
    onesb_d = nc.declare_dram_parameter("ones_b", [128, 1], bf16, isOutput=False)
    meanb_d = nc.declare_dram_parameter("mean_b", [128, 1], bf16, isOutput=False)
    out_d = nc.declare_dram_parameter("out", [BL, L], f32, isOutput=True)

    with tile.TileContext(nc) as tc:
        with (
            tc.tile_pool(name="const", bufs=1) as cpool,
            tc.tile_pool(name="lhp", bufs=6) as lhpool,
            tc.tile_pool(name="scr", bufs=2) as scpool,
            tc.tile_pool(name="work", bufs=3) as wpool,
            tc.tile_pool(name="small", bufs=6) as spool,
            tc.tile_pool(name="psA", bufs=2, space="PSUM") as psA,
            tc.tile_pool(name="psB", bufs=3, space="PSUM") as psB,
            tc.tile_pool(name="psC", bufs=1, space="PSUM") as psC,
        ):
            # ---- constants (loaded once) ----
            iota_f = cpool.tile([128, 128], f32)
            nc.sync.dma_start(iota_f[:], iotaf_d[:])
            ident_b = cpool.tile([128, 128], bf16)
            nc.sync.dma_start(ident_b[:], identb_d[:])
            ident_f = cpool.tile([128, 128], f32)
            nc.sync.dma_start(ident_f[:], identf_d[:])
            wrb = cpool.tile([128, H], f32)
            nc.sync.dma_start(wrb[:], wrb_d[:])
            w1s = cpool.tile([128, HC, GH], bf16)
            nc.sync.dma_start(w1s[:], w1t_d[:])
            w2s = cpool.tile([GH, GH], bf16)
            nc.sync.dma_start(w2s[:], w2t_d[:])
            wf1s = cpool.tile([128, FC, FH], f32)
            nc.sync.dma_start(wf1s[:], wf1t_d[:])
            wf2s = cpool.tile([128, 2, L], f32)
            nc.sync.dma_start(wf2s[:], wf2t_d[:])
            b1b = cpool.tile([128, GH], f32)
            nc.sync.dma_start(b1b[:], b1b_d[:])
            b2b = cpool.tile([128, GH], f32)
            nc.sync.dma_start(b2b[:], b2b_d[:])
            bf1b = cpool.tile([BL, FH], f32)
            nc.sync.dma_start(bf1b[:], bf1b_d[:])
            bf2b = cpool.tile([BL, L], f32)
            nc.sync.dma_start(bf2b[:], bf2b_d[:])
            ones_r = cpool.tile([128, 1], f32r)
            nc.sync.dma_start(ones_r[:], onesr_d[:])
            ones_b = cpool.tile([128, 1], bf16)
            nc.sync.dma_start(ones_b[:], onesb_d[:])
            mean_b = cpool.tile([128, 1], bf16)
            nc.sync.dma_start(mean_b[:], meanb_d[:])
            # pooled graph embeddings (written one column per graph)
            catT6 = cpool.tile([128, BL], f32)

            # ---------- phase 0: adjacency + degrees for all graphs ----------
            atis = []
            dinvs = []
            subvs = []
            for g in range(BL):
                subv = spool.tile([128, SC], f32, tag="subv", bufs=BL)
                nc.sync.dma_start(subv[:], subv_d[g])
                subvs.append(subv)
                esrc = spool.tile([128, EC], f32, tag="esrc", bufs=2)
                nc.sync.dma_start(esrc[:], esrc_d[g])
                edst = spool.tile([128, EC], f32, tag="edst", bufs=2)
                nc.sync.dma_start(edst[:], edst_d[g])

                at_ps = psB.tile([128, 128], f32, tag="mm")
                for c in range(EC):
                    s_t = wpool.tile([128, 128], bf16, tag="ohS")
                    d_t = wpool.tile([128, 128], bf16, tag="ohD")
                    nc.vector.tensor_scalar(s_t[:], iota_f[:], esrc[:, c : c + 1],
                                            None, ALU.is_equal)
                    nc.vector.tensor_scalar(d_t[:], iota_f[:], edst[:, c : c + 1],
                                            None, ALU.is_equal)
                    nc.tensor.matmul(at_ps[:], s_t[:], d_t[:], start=(c == 0),
                                     stop=False)
                # += I (self-loops) via identity outer product, exact in bf16
                nc.tensor.matmul(at_ps[:], ident_b[:], ident_b[:], start=False,
                                 stop=True)
                ati = wpool.tile([128, 128], bf16, tag="ati", bufs=BL)
                nc.scalar.copy(ati[:], at_ps[:])
                atis.append(ati)
                # deg[d] = sum_s ATI[s,d]  -> dinv = 1/sqrt(deg)
                deg_ps = psB.tile([128, 1], f32, tag="mm")
                nc.tensor.matmul(deg_ps[:], ati[:], ones_b[:],
                                 start=True, stop=True)
                sdeg = spool.tile([128, 1], f32, tag="sv")
                nc.scalar.activation(sdeg[:], deg_ps[:], AFT.Sqrt)
                dinv = spool.tile([128, 1], f32, tag="dinv", bufs=BL)
                nc.vector.reciprocal(dinv[:], sdeg[:])
                dinvs.append(dinv)

            # ---------- phase 1: gate + pooling + GCN per graph ----------
            for g in range(BL):
                subv = subvs[g]
                ati = atis[g]
                dinv = dinvs[g]

                cnt_ps = psC.tile([128, SC], f32, tag="v")
                nf_ps = psA.tile([128, H], f32, tag="nf")
                for c in range(SC):
                    lht = lhpool.tile([128, H], f32r, tag="lh")
                    nc.sync.dma_start(lht[:], lh_d[g, c * 128 : (c + 1) * 128, :])
                    scr = scpool.tile([128, H], bf16, tag="scr")
                    logits = spool.tile([128, 1], f32, tag="sv")
                    nc.vector.scalar_tensor_tensor(
                        scr[:], lht[:].bitcast(f32), 0.0, wrb[:], ALU.bypass,
                        ALU.mult, accum_out=logits[:])
                    gate = spool.tile([128, 1], f32, tag="sv")
                    nc.scalar.activation(gate[:], logits[:], AFT.Sigmoid,
                                         bias=float(br_val))
                    p_t = wpool.tile([128, 128], bf16, tag="ohP")
                    nc.vector.tensor_scalar(p_t[:], iota_f[:], subv[:, c : c + 1],
                                            None, ALU.is_equal)
                    pg_t = wpool.tile([128, 128], f32r, tag="ohPg")
                    nc.scalar.mul(pg_t[:], p_t[:], gate[:])
                    nc.tensor.matmul(cnt_ps[:, c : c + 1], p_t[:], ones_b[:],
                                     start=True, stop=True)
                    # pooled node feats: nf[n,h] += Pg[s,n]^T lh[s,h]
                    nc.tensor.matmul(nf_ps[:, 0:512], pg_t[:], lht[:, 0:512],
                                     start=(c == 0), stop=(c == SC - 1))
                    nc.tensor.matmul(nf_ps[:, 512:H], pg_t[:], lht[:, 512:H],
                                     start=(c == 0), stop=(c == SC - 1))

                # 1/max(cnt,1); combined layer-1 row scale s1 = invc * dinv
                cnt1 = spool.tile([128, 1], f32, tag="sv")
                nc.vector.tensor_reduce(cnt1[:], cnt_ps[:], mybir.AxisListType.X,
                                        ALU.add)
                mx = spool.tile([128, 1], f32, tag="sv")
                nc.vector.tensor_scalar_max(mx[:], cnt1[:], 1.0)
                invc = spool.tile([128, 1], f32, tag="sv")
                nc.vector.reciprocal(invc[:], mx[:])
                s1 = spool.tile([128, 1], f32, tag="sv")
                nc.vector.tensor_tensor(s1[:], invc[:], dinv[:], ALU.mult)

                # scale rows by s1 while moving PSUM->SBUF (bf16 for layer 1)
                nf_sb = wpool.tile([128, H], bf16, tag="nfsb", bufs=2)
                nc.vector.tensor_scalar_mul(nf_sb[:], nf_ps[:], s1[:])
                # transpose to nfT chunks [h,n]
                nfs = wpool.tile([128, HC, GH], bf16, tag="nfs", bufs=2)
                for hc in range(HC):
                    tr_ps = psB.tile([128, 128], bf16, tag="mm")
                    nc.tensor.transpose(tr_ps[:], nf_sb[:, hc * 128 : (hc + 1) * 128],
                                        ident_b[:])
                    nc.vector.tensor_copy(nfs[:, hc, :], tr_ps[:])

                # GCN layer 1: T2 = (s1*sums) @ W1  (scale pre-applied)
                t1_ps = psB.tile([128, GH], f32, tag="mm")
                for hc in range(HC):
                    nc.tensor.matmul(t1_ps[:], nfs[:, hc, :], w1s[:, hc, :],
                                     start=(hc == 0), stop=(hc == HC - 1))
                t2 = wpool.tile([128, GH], bf16, tag="t2")
                nc.vector.tensor_copy(t2[:], t1_ps[:])
                z_ps = psB.tile([128, GH], f32, tag="mm")
                nc.tensor.matmul(z_ps[:], ati[:], t2[:], start=True, stop=True)
                x1 = wpool.tile([128, GH], bf16, tag="x1")
                if b1_zero:
                    # x1 = dinv * relu(z)  (valid since dinv > 0)
                    nc.vector.tensor_scalar(x1[:], z_ps[:], 0.0, dinv[:],
                                            ALU.max, ALU.mult)
                else:
                    x1p = wpool.tile([128, GH], f32, tag="x1p")
                    nc.vector.scalar_tensor_tensor(x1p[:], z_ps[:], dinv[:],
                                                   b1b[:], ALU.mult, ALU.add)
                    nc.vector.tensor_scalar_max(x1[:], x1p[:], 0.0)

                # GCN layer 2
                x1t_ps = psB.tile([128, GH], bf16, tag="mm")
                nc.tensor.transpose(x1t_ps[:], x1[:], ident_b[:])
                x1t = wpool.tile([128, GH], bf16, tag="x1t")
                nc.vector.tensor_copy(x1t[:], x1t_ps[:])
                tp_ps = psB.tile([128, GH], f32, tag="mm")
                nc.tensor.matmul(tp_ps[:], x1t[:], w2s[:], start=True, stop=True)
                t2p = wpool.tile([128, GH], bf16, tag="t2")
                nc.vector.tensor_scalar_mul(t2p[:], tp_ps[:], dinv[:])
                z2_ps = psB.tile([128, GH], f32, tag="mm")
                nc.tensor.matmul(z2_ps[:], ati[:], t2p[:], start=True, stop=True)
                x2 = wpool.tile([128, GH], bf16, tag="x1")
                if b2_zero:
                    nc.vector.tensor_scalar(x2[:], z2_ps[:], 0.0, dinv[:],
                                            ALU.max, ALU.mult)
                else:
                    x2p = wpool.tile([128, GH], f32, tag="x1p")
                    nc.vector.scalar_tensor_tensor(x2p[:], z2_ps[:], dinv[:],
                                                   b2b[:], ALU.mult, ALU.add)
                    nc.vector.tensor_scalar_max(x2[:], x2p[:], 0.0)

                # graph mean pool -> column g of catT6
                pool_ps = psC.tile([128, 1], f32, tag="v")
                nc.tensor.matmul(pool_ps[:], x2[:], mean_b[:], start=True,
                                 stop=True)
                nc.scalar.copy(catT6[:, g : g + 1], pool_ps[:])

            # ---------- FC head over all BL graphs ----------
            clsr = cpool.tile([BL, H], f32)
            nc.sync.dma_start(clsr[:], lh_d[:, 0, :].bitcast(f32))
            h1_ps = psB.tile([BL, FH], f32, tag="mm")
            for c in range(FC):
                if c < HC:
                    ct_ps = psB.tile([128, BL], f32, tag="mm")
                    nc.tensor.transpose(ct_ps[:], clsr[:, c * 128 : (c + 1) * 128],
                                        ident_f[0:BL, 0:BL])
                    catc = wpool.tile([128, BL], f32, tag="catc", bufs=2)
                    nc.vector.tensor_copy(catc[:], ct_ps[:])
                else:
                    catc = catT6
                nc.tensor.matmul(h1_ps[:], catc[:], wf1s[:, c, :], start=(c == 0),
                                 stop=(c == FC - 1))
            h1s = wpool.tile([BL, FH], f32, tag="h1")
            nc.vector.scalar_tensor_tensor(h1s[:], h1_ps[:], 1.0, bf1b[:],
                                           ALU.bypass, ALU.add)
            hr = wpool.tile([BL, FH], f32, tag="h1")
            nc.vector.tensor_scalar_max(hr[:], h1s[:], 0.0)
            out_ps = psC.tile([BL, L], f32, tag="v")
            for c in range(2):
                ht_ps = psB.tile([128, BL], f32, tag="mm")
                nc.tensor.transpose(ht_ps[:], hr[:, c * 128 : (c + 1) * 128],
                                    ident_f[0:BL, 0:BL])
                htc = wpool.tile([128, BL], f32, tag="catc", bufs=2)
                nc.vector.tensor_copy(htc[:], ht_ps[:])
                nc.tensor.matmul(out_ps[:], htc[:], wf2s[:, c, :], start=(c == 0),
                                 stop=(c == 1))
            outs = wpool.tile([BL, L], f32, tag="outs")
            nc.vector.scalar_tensor_tensor(outs[:], out_ps[:], 1.0, bf2b[:],
                                           ALU.bypass, ALU.add)
            nc.sync.dma_start(out_d[:], outs[:])

    _split_multi_waits(nc)
    return nc


def _prepare_in_maps(inputs):
    lh = np.ascontiguousarray(np.asarray(inputs["last_hidden"], dtype=np.float32))
    submap = np.asarray(inputs["submap"]).astype(np.int64)
    edge_index = np.asarray(inputs["edge_index"]).astype(np.int64)
    assert lh.shape == (B, S, H)
    assert int(inputs.get("num_nodes", N)) == N

    wr = np.asarray(inputs["wr"], dtype=np.float32)
    br = float(np.asarray(inputs["br"], dtype=np.float32))
    W1 = np.asarray(inputs["W1"], dtype=np.float32)
    b1 = np.asarray(inputs["b1"], dtype=np.float32)
    W2 = np.asarray(inputs["W2"], dtype=np.float32)
    b2 = np.asarray(inputs["b2"], dtype=np.float32)
    Wf1 = np.asarray(inputs["Wf1"], dtype=np.float32)
    bf1 = np.asarray(inputs["bf1"], dtype=np.float32)
    Wf2 = np.asarray(inputs["Wf2"], dtype=np.float32)
    bf2 = np.asarray(inputs["bf2"], dtype=np.float32)

    # Shared (replicated) tensors.
    consts = {
        "wrb": np.ascontiguousarray(np.broadcast_to(wr, (128, H))),
        "w1t": np.ascontiguousarray(
            W1.reshape(HC, 128, GH).transpose(1, 0, 2)).astype(BF16),
        "w2t": np.ascontiguousarray(W2).astype(BF16),
        "wf1t": np.ascontiguousarray(
            Wf1.reshape(FC, 128, FH).transpose(1, 0, 2)),
        "wf2t": np.ascontiguousarray(
            Wf2.reshape(2, 128, L).transpose(1, 0, 2)),
        "b1b": np.ascontiguousarray(np.broadcast_to(b1, (128, GH))),
        "b2b": np.ascontiguousarray(np.broadcast_to(b2, (128, GH))),
        "bf1b": np.ascontiguousarray(np.broadcast_to(bf1, (BL, FH))),
        "bf2b": np.ascontiguousarray(np.broadcast_to(bf2, (BL, L))),
        "iota_f": np.ascontiguousarray(
            np.broadcast_to(np.arange(128, dtype=np.float32), (128, 128))),
        "ident_b": np.eye(128, dtype=np.float32).astype(BF16),
        "ident_f": np.eye(128, dtype=np.float32),
        "ones_r": np.ones((128, 1), np.float32),
        "ones_b": np.ones((128, 1), np.float32).astype(BF16),
        "mean_b": np.full((128, 1), 1.0 / N, np.float32).astype(BF16),
    }

    # Per-graph index layouts: value of token t goes to partition t%128,
    # column t//128.
    subv = submap.reshape(B, SC, 128).transpose(0, 2, 1).astype(np.float32)
    esrc = edge_index[:, 0, :].reshape(B, EC, 128).transpose(0, 2, 1).astype(np.float32)
    edst = edge_index[:, 1, :].reshape(B, EC, 128).transpose(0, 2, 1).astype(np.float32)

    in_maps = []
    for i in range(NCORES):
        sl = slice(i * BL, (i + 1) * BL)
        m = dict(consts)
        m["lh"] = np.ascontiguousarray(lh[sl])
        m["subv"] = np.ascontiguousarray(subv[sl])
        m["esrc"] = np.ascontiguousarray(esrc[sl])
        m["edst"] = np.ascontiguousarray(edst[sl])
        in_maps.append(m)
    flags = (br, bool(np.all(b1 == 0)), bool(np.all(b2 == 0)))
    return in_maps, flags


def _run(inputs, trace=False):
    in_maps, flags = _prepare_in_maps(inputs)
    key = ("prog",) + flags
    if key not in _CACHE:
        _CACHE[key] = build_program(*flags)
    nc = _CACHE[key]
    res = run_bass_kernel_spmd(nc, in_maps, list(range(NCORES)), trace=trace)
    out = np.concatenate([np.asarray(res.results[i]["out"]) for i in range(NCORES)],
                         axis=0).astype(np.float32)
    return out, res


def kernel(**inputs) -> np.ndarray:
    out, _ = _run(inputs, trace=False)
    return out


# revision 13
# speedup vs baseline: 2.2812x; 1.0242x over previous
"""Trainium2 Bass kernel for BioBERT-ARG-GNN (gated pooling + 2-layer GCN + MLP head).

Strategy: pure data parallel over batch B=64 across 8 NeuronCores (8 graphs
per core).  All segment/gather ops are dense matmuls against one-hot
matrices built on-device from the index tensors (N=128 nodes == partition
dim).  GCN normalization (D^-1/2 (A+I) D^-1/2) factors into per-partition
scalings around a dense [128,128] adjacency matmul.  Matmul dtypes: f32r
(TF32-like, 1 cycle/row at free-dim>=256) for the big subtoken pooling,
bf16 for the [128,128] GCN matmuls (adjacency counts are exact), f32 for
the tiny FC head.  Phase split keeps each ACT function's table loaded once.
"""

import os
import sys

import numpy as np

for _p in ("/opt/trn_rl_repo", "/root/.axon_site/_ro/trn_rl_repo"):
    if os.path.isdir(_p) and _p not in sys.path:
        sys.path.insert(0, _p)

import ml_dtypes  # noqa: E402
import concourse.bass as bass  # noqa: E402
import concourse.mybir as mybir  # noqa: E402
from concourse import tile  # noqa: E402
from concourse.bass_utils import run_bass_kernel_spmd  # noqa: E402

# Problem shapes (hardcoded per contest rules).
B, S, H = 64, 512, 768
N, E = 128, 1024
GH, FH, L = 128, 256, 2
NCORES = 8
BL = B // NCORES  # graphs per core
SC = S // 128     # subtoken chunks per graph
EC = E // 128     # edge chunks per graph
HC = H // 128     # BERT-hidden chunks
FC = (H + GH) // 128  # concat-feature chunks for the FC head

f32 = mybir.dt.float32
f32r = mybir.dt.float32r
bf16 = mybir.dt.bfloat16
AFT = mybir.ActivationFunctionType
ALU = mybir.AluOpType
BF16 = ml_dtypes.bfloat16

_CACHE = {}


def _split_multi_waits(nc: bass.Bass) -> int:
    """Walrus in this container accepts one sync-wait per instruction; split
    extra waits into single-wait EventSemaphore nops just before it."""
    n_split = 0
    for fn in nc.m.functions:
        for blk in fn.blocks:
            new_instrs = []
            changed = False
            for inst in blk.instructions:
                si = getattr(inst, "sync_info", None)
                if si is not None and si.on_wait is not None and len(si.on_wait) > 1:
                    waits = list(si.on_wait)
                    for j, w in enumerate(waits[:-1]):
                        ev = mybir.InstEventSemaphore(
                            name=f"{inst.name}_ws{j}",
                            ins=[], outs=[],
                            engine=inst.engine,
                            sync_info=mybir.SyncInfo(on_wait=[w], on_update=[]),
                        )
                        new_instrs.append(ev)
                    inst.sync_info = mybir.SyncInfo(
                        on_wait=[waits[-1]], on_update=list(si.on_update))
                    n_split += 1
                    changed = True
                new_instrs.append(inst)
            if changed:
                blk.instructions = new_instrs
    return n_split


def build_program(br_val: float, b1_zero: bool, b2_zero: bool) -> bass.Bass:
    nc = bass.Bass()

    lh_d = nc.declare_dram_parameter("lh", [BL, S, H], f32r, isOutput=False)
    subv_d = nc.declare_dram_parameter("subv", [BL, 128, SC], f32, isOutput=False)
    esrc_d = nc.declare_dram_parameter("esrc", [BL, 128, EC], f32, isOutput=False)
    edst_d = nc.declare_dram_parameter("edst", [BL, 128, EC], f32, isOutput=False)
    wrb_d = nc.declare_dram_parameter("wrb", [128, H], f32, isOutput=False)
    w1t_d = nc.declare_dram_parameter("w1t", [128, HC, GH], bf16, isOutput=False)
    w2t_d = nc.declare_dram_parameter("w2t", [GH, GH], bf16, isOutput=False)
    wf1t_d = nc.declare_dram_parameter("wf1t", [128, FC, FH], f32, isOutput=False)
    wf2t_d = nc.declare_dram_parameter("wf2t", [128, 2, L], f32, isOutput=False)
    b1b_d = nc.declare_dram_parameter("b1b", [128, GH], f32, isOutput=False)
    b2b_d = nc.declare_dram_parameter("b2b", [128, GH], f32, isOutput=False)
    bf1b_d = nc.declare_dram_parameter("bf1b", [BL, FH], f32, isOutput=False)
    bf2b_d = nc.declare_dram_parameter("bf2b", [BL, L], f32, isOutput=False)
    iotaf_d = nc.declare_dram_parameter("iota_f", [128, 128], f32, isOutput=False)
    iota8_d = nc.declare_dram_parameter("iota8", [128, EC, 128], f32, isOutput=False)
    identb_d = nc.declare_dram_parameter("ident_b", [128, 128], bf16, isOutput=False)
    identf_d = nc.declare_dram_parameter("ident_f", [128, 128], f32, isOutput=False)
    onesr_d = nc.declare_dram_parameter("ones_r", [128, 1], f32r, isOutput=False)
    onesb_d = nc.declare_dram_parameter("ones_b", [128, 1], bf16, isOutput=False)
    meanb_d = nc.declare_dram_parameter("mean_b", [128, 1], bf16, isOutput=False)
    out_d = nc.declare_dram_parameter("out", [BL, L], f32, isOutput=True)

    with tile.TileContext(nc) as tc:
        with (
            tc.tile_pool(name="const", bufs=1) as cpool,
            tc.tile_pool(name="lhp", bufs=6) as lhpool,
            tc.tile_pool(name="scr", bufs=2) as scpool,
            tc.tile_pool(name="work", bufs=3) as wpool,
            tc.tile_pool(name="small", bufs=6) as spool,
            tc.tile_pool(name="psA", bufs=2, space="PSUM") as psA,
            tc.tile_pool(name="psB", bufs=3, space="PSUM") as psB,
            tc.tile_pool(name="psC", bufs=1, space="PSUM") as psC,
        ):
            # ---- constants (loaded once) ----
            iota_f = cpool.tile([128, 128], f32)
            nc.sync.dma_start(iota_f[:], iotaf_d[:])
            iota8 = cpool.tile([128, EC, 128], f32)
            nc.sync.dma_start(iota8[:], iota8_d[:])
            ident_b = cpool.tile([128, 128], bf16)
            nc.sync.dma_start(ident_b[:], identb_d[:])
            ident_f = cpool.tile([128, 128], f32)
            nc.sync.dma_start(ident_f[:], identf_d[:])
            wrb = cpool.tile([128, H], f32)
            nc.sync.dma_start(wrb[:], wrb_d[:])
            w1s = cpool.tile([128, HC, GH], bf16)
            nc.sync.dma_start(w1s[:], w1t_d[:])
            w2s = cpool.tile([GH, GH], bf16)
            nc.sync.dma_start(w2s[:], w2t_d[:])
            wf1s = cpool.tile([128, FC, FH], f32)
            nc.sync.dma_start(wf1s[:], wf1t_d[:])
            wf2s = cpool.tile([128, 2, L], f32)
            nc.sync.dma_start(wf2s[:], wf2t_d[:])
            b1b = cpool.tile([128, GH], f32)
            nc.sync.dma_start(b1b[:], b1b_d[:])
            b2b = cpool.tile([128, GH], f32)
            nc.sync.dma_start(b2b[:], b2b_d[:])
            bf1b = cpool.tile([BL, FH], f32)
            nc.sync.dma_start(bf1b[:], bf1b_d[:])
            bf2b = cpool.tile([BL, L], f32)
            nc.sync.dma_start(bf2b[:], bf2b_d[:])
            ones_r = cpool.tile([128, 1], f32r)
            nc.sync.dma_start(ones_r[:], onesr_d[:])
            ones_b = cpool.tile([128, 1], bf16)
            nc.sync.dma_start(ones_b[:], onesb_d[:])
            mean_b = cpool.tile([128, 1], bf16)
            nc.sync.dma_start(mean_b[:], meanb_d[:])
            # pooled graph embeddings (written one column per graph)
            catT6 = cpool.tile([128, BL], f32)

            # ---------- phase 0: adjacency + degrees for all graphs ----------
            atis = []
            dinvs = []
            subvs = []
            for g in range(BL):
                subv = spool.tile([128, SC], f32, tag="subv", bufs=BL)
                nc.sync.dma_start(subv[:], subv_d[g])
                subvs.append(subv)
                esrc = spool.tile([128, EC], f32, tag="esrc", bufs=2)
                nc.sync.dma_start(esrc[:], esrc_d[g])
                edst = spool.tile([128, EC], f32, tag="edst", bufs=2)
                nc.sync.dma_start(edst[:], edst_d[g])

                at_ps = psB.tile([128, 128], f32, tag="mm")
                s_all = wpool.tile([128, EC, 128], bf16, tag="ohS")
                nc.vector.tensor_tensor(
                    out=s_all[:], in0=esrc[:].broadcast_to([128, EC, 128]),
                    in1=iota8[:], op=ALU.is_equal)
                d_all = wpool.tile([128, EC, 128], bf16, tag="ohD")
                nc.vector.tensor_tensor(
                    out=d_all[:], in0=edst[:].broadcast_to([128, EC, 128]),
                    in1=iota8[:], op=ALU.is_equal)
                for c in range(EC):
                    nc.tensor.matmul(at_ps[:], s_all[:, c, :], d_all[:, c, :],
                                     start=(c == 0), stop=False)
                # += I (self-loops) via identity outer product, exact in bf16
                nc.tensor.matmul(at_ps[:], ident_b[:], ident_b[:], start=False,
                                 stop=True)
                ati = wpool.tile([128, 128], bf16, tag="ati", bufs=BL)
                nc.scalar.copy(ati[:], at_ps[:])
                atis.append(ati)
                # deg[d] = sum_s ATI[s,d]  -> dinv = 1/sqrt(deg)
                deg_ps = psB.tile([128, 1], f32, tag="mm")
                nc.tensor.matmul(deg_ps[:], ati[:], ones_b[:],
                                 start=True, stop=True)
                sdeg = spool.tile([128, 1], f32, tag="sv")
                nc.scalar.activation(sdeg[:], deg_ps[:], AFT.Sqrt)
                dinv = spool.tile([128, 1], f32, tag="dinv", bufs=BL)
                nc.vector.reciprocal(dinv[:], sdeg[:])
                dinvs.append(dinv)

            # ---------- phase 1: gate + pooling + GCN per graph ----------
            for g in range(BL):
                subv = subvs[g]
                ati = atis[g]
                dinv = dinvs[g]

                cnt_ps = psC.tile([128, SC], f32, tag="v")
                nf_ps = psA.tile([128, H], f32, tag="nf")
                p_all = wpool.tile([128, SC, 128], bf16, tag="ohP")
                nc.vector.tensor_tensor(
                    out=p_all[:], in0=subv[:].broadcast_to([128, SC, 128]),
                    in1=iota8[:, 0:SC, :], op=ALU.is_equal)
                for c in range(SC):
                    lht = lhpool.tile([128, H], f32r, tag="lh")
                    nc.sync.dma_start(lht[:], lh_d[g, c * 128 : (c + 1) * 128, :])
                    scr = scpool.tile([128, H], bf16, tag="scr")
                    logits = spool.tile([128, 1], f32, tag="sv")
                    nc.vector.scalar_tensor_tensor(
                        scr[:], lht[:].bitcast(f32), 0.0, wrb[:], ALU.bypass,
                        ALU.mult, accum_out=logits[:])
                    gate = spool.tile([128, 1], f32, tag="sv")
                    nc.scalar.activation(gate[:], logits[:], AFT.Sigmoid,
                                         bias=float(br_val))
                    pg_t = wpool.tile([128, 128], f32r, tag="ohPg")
                    nc.scalar.mul(pg_t[:], p_all[:, c, :], gate[:])
                    nc.tensor.matmul(cnt_ps[:, c : c + 1], p_all[:, c, :],
                                     ones_b[:], start=True, stop=True)
                    # pooled node feats: nf[n,h] += Pg[s,n]^T lh[s,h]
                    nc.tensor.matmul(nf_ps[:, 0:512], pg_t[:], lht[:, 0:512],
                                     start=(c == 0), stop=(c == SC - 1))
                    nc.tensor.matmul(nf_ps[:, 512:H], pg_t[:], lht[:, 512:H],
                                     start=(c == 0), stop=(c == SC - 1))

                # 1/max(cnt,1); combined layer-1 row scale s1 = invc * dinv
                cnt1 = spool.tile([128, 1], f32, tag="sv")
                nc.vector.tensor_reduce(cnt1[:], cnt_ps[:], mybir.AxisListType.X,
                                        ALU.add)
                mx = spool.tile([128, 1], f32, tag="sv")
                nc.vector.tensor_scalar_max(mx[:], cnt1[:], 1.0)
                invc = spool.tile([128, 1], f32, tag="sv")
                nc.vector.reciprocal(invc[:], mx[:])
                s1 = spool.tile([128, 1], f32, tag="sv")
                nc.vector.tensor_tensor(s1[:], invc[:], dinv[:], ALU.mult)

                # scale rows by s1 while moving PSUM->SBUF (bf16 for layer 1)
                nf_sb = wpool.tile([128, H], bf16, tag="nfsb", bufs=2)
                nc.vector.tensor_scalar_mul(nf_sb[:], nf_ps[:], s1[:])
                # transpose to nfT chunks [h,n]
                nfs = wpool.tile([128, HC, GH], bf16, tag="nfs", bufs=2)
                for hc in range(HC):
                    tr_ps = psB.tile([128, 128], bf16, tag="mm")
                    nc.tensor.transpose(tr_ps[:], nf_sb[:, hc * 128 : (hc + 1) * 128],
                                        ident_b[:])
                    nc.any.tensor_copy(nfs[:, hc, :], tr_ps[:])

                # GCN layer 1: T2 = (s1*sums) @ W1  (scale pre-applied)
                t1_ps = psB.tile([128, GH], f32, tag="mm")
                for hc in range(HC):
                    nc.tensor.matmul(t1_ps[:], nfs[:, hc, :], w1s[:, hc, :],
                                     start=(hc == 0), stop=(hc == HC - 1))
                t2 = wpool.tile([128, GH], bf16, tag="t2")
                nc.any.tensor_copy(t2[:], t1_ps[:])
                z_ps = psB.tile([128, GH], f32, tag="mm")
                nc.tensor.matmul(z_ps[:], ati[:], t2[:], start=True, stop=True)
                x1 = wpool.tile([128, GH], bf16, tag="x1")
                if b1_zero:
                    # x1 = dinv * relu(z)  (valid since dinv > 0)
                    nc.vector.tensor_scalar(x1[:], z_ps[:], 0.0, dinv[:],
                                            ALU.max, ALU.mult)
                else:
                    x1p = wpool.tile([128, GH], f32, tag="x1p")
                    nc.vector.scalar_tensor_tensor(x1p[:], z_ps[:], dinv[:],
                                                   b1b[:], ALU.mult, ALU.add)
                    nc.vector.tensor_scalar_max(x1[:], x1p[:], 0.0)

                # GCN layer 2
                x1t_ps = psB.tile([128, GH], bf16, tag="mm")
                nc.tensor.transpose(x1t_ps[:], x1[:], ident_b[:])
                x1t = wpool.tile([128, GH], bf16, tag="x1t")
                nc.any.tensor_copy(x1t[:], x1t_ps[:])
                tp_ps = psB.tile([128, GH], f32, tag="mm")
                nc.tensor.matmul(tp_ps[:], x1t[:], w2s[:], start=True, stop=True)
                t2p = wpool.tile([128, GH], bf16, tag="t2")
                nc.vector.tensor_scalar_mul(t2p[:], tp_ps[:], dinv[:])
                z2_ps = psB.tile([128, GH], f32, tag="mm")
                nc.tensor.matmul(z2_ps[:], ati[:], t2p[:], start=True, stop=True)
                x2 = wpool.tile([128, GH], bf16, tag="x1")
                if b2_zero:
                    nc.vector.tensor_scalar(x2[:], z2_ps[:], 0.0, dinv[:],
                                            ALU.max, ALU.mult)
                else:
                    x2p = wpool.tile([128, GH], f32, tag="x1p")
                    nc.vector.scalar_tensor_tensor(x2p[:], z2_ps[:], dinv[:],
                                                   b2b[:], ALU.mult, ALU.add)
                    nc.vector.tensor_scalar_max(x2[:], x2p[:], 0.0)

                # graph mean pool -> column g of catT6
                pool_ps = psC.tile([128, 1], f32, tag="v")
                nc.tensor.matmul(pool_ps[:], x2[:], mean_b[:], start=True,
                                 stop=True)
                nc.scalar.copy(catT6[:, g : g + 1], pool_ps[:])

            # ---------- FC head over all BL graphs ----------
            clsr = cpool.tile([BL, H], f32)
            nc.sync.dma_start(clsr[:], lh_d[:, 0, :].bitcast(f32))
            h1_ps = psB.tile([BL, FH], f32, tag="mm")
            for c in range(FC):
                if c < HC:
                    ct_ps = psB.tile([128, BL], f32, tag="mm")
                    nc.tensor.transpose(ct_ps[:], clsr[:, c * 128 : (c + 1) * 128],
                                        ident_f[0:BL, 0:BL])
                    catc = wpool.tile([128, BL], f32, tag="catc", bufs=2)
                    nc.any.tensor_copy(catc[:], ct_ps[:])
                else:
                    catc = catT6
                nc.tensor.matmul(h1_ps[:], catc[:], wf1s[:, c, :], start=(c == 0),
                                 stop=(c == FC - 1))
            h1s = wpool.tile([BL, FH], f32, tag="h1")
            nc.vector.scalar_tensor_tensor(h1s[:], h1_ps[:], 1.0, bf1b[:],
                                           ALU.bypass, ALU.add)
            hr = wpool.tile([BL, FH], f32, tag="h1")
            nc.vector.tensor_scalar_max(hr[:], h1s[:], 0.0)
            out_ps = psC.tile([BL, L], f32, tag="v")
            for c in range(2):
                ht_ps = psB.tile([128, BL], f32, tag="mm")
                nc.tensor.transpose(ht_ps[:], hr[:, c * 128 : (c + 1) * 128],
                                    ident_f[0:BL, 0:BL])
                htc = wpool.tile([128, BL], f32, tag="catc", bufs=2)
                nc.any.tensor_copy(htc[:], ht_ps[:])
                nc.tensor.matmul(out_ps[:], htc[:], wf2s[:, c, :], start=(c == 0),
                                 stop=(c == 1))
            outs = wpool.tile([BL, L], f32, tag="outs")
            nc.vector.scalar_tensor_tensor(outs[:], out_ps[:], 1.0, bf2b[:],
                                           ALU.bypass, ALU.add)
            nc.sync.dma_start(out_d[:], outs[:])

    _split_multi_waits(nc)
    return nc


def _prepare_in_maps(inputs):
    lh = np.ascontiguousarray(np.asarray(inputs["last_hidden"], dtype=np.float32))
    submap = np.asarray(inputs["submap"]).astype(np.int64)
    edge_index = np.asarray(inputs["edge_index"]).astype(np.int64)
    assert lh.shape == (B, S, H)
    assert int(inputs.get("num_nodes", N)) == N

    wr = np.asarray(inputs["wr"], dtype=np.float32)
    br = float(np.asarray(inputs["br"], dtype=np.float32))
    W1 = np.asarray(inputs["W1"], dtype=np.float32)
    b1 = np.asarray(inputs["b1"], dtype=np.float32)
    W2 = np.asarray(inputs["W2"], dtype=np.float32)
    b2 = np.asarray(inputs["b2"], dtype=np.float32)
    Wf1 = np.asarray(inputs["Wf1"], dtype=np.float32)
    bf1 = np.asarray(inputs["bf1"], dtype=np.float32)
    Wf2 = np.asarray(inputs["Wf2"], dtype=np.float32)
    bf2 = np.asarray(inputs["bf2"], dtype=np.float32)

    # Shared (replicated) tensors.
    consts = {
        "wrb": np.ascontiguousarray(np.broadcast_to(wr, (128, H))),
        "w1t": np.ascontiguousarray(
            W1.reshape(HC, 128, GH).transpose(1, 0, 2)).astype(BF16),
        "w2t": np.ascontiguousarray(W2).astype(BF16),
        "wf1t": np.ascontiguousarray(
            Wf1.reshape(FC, 128, FH).transpose(1, 0, 2)),
        "wf2t": np.ascontiguousarray(
            Wf2.reshape(2, 128, L).transpose(1, 0, 2)),
        "b1b": np.ascontiguousarray(np.broadcast_to(b1, (128, GH))),
        "b2b": np.ascontiguousarray(np.broadcast_to(b2, (128, GH))),
        "bf1b": np.ascontiguousarray(np.broadcast_to(bf1, (BL, FH))),
        "bf2b": np.ascontiguousarray(np.broadcast_to(bf2, (BL, L))),
        "iota_f": np.ascontiguousarray(
            np.broadcast_to(np.arange(128, dtype=np.float32), (128, 128))),
        "iota8": np.ascontiguousarray(
            np.broadcast_to(np.arange(128, dtype=np.float32), (128, EC, 128))),
        "ident_b": np.eye(128, dtype=np.float32).astype(BF16),
        "ident_f": np.eye(128, dtype=np.float32),
        "ones_r": np.ones((128, 1), np.float32),
        "ones_b": np.ones((128, 1), np.float32).astype(BF16),
        "mean_b": np.full((128, 1), 1.0 / N, np.float32).astype(BF16),
    }

    # Per-graph index layouts: value of token t goes to partition t%128,
    # column t//128.
    subv = submap.reshape(B, SC, 128).transpose(0, 2, 1).astype(np.float32)
    esrc = edge_index[:, 0, :].reshape(B, EC, 128).transpose(0, 2, 1).astype(np.float32)
    edst = edge_index[:, 1, :].reshape(B, EC, 128).transpose(0, 2, 1).astype(np.float32)

    in_maps = []
    for i in range(NCORES):
        sl = slice(i * BL, (i + 1) * BL)
        m = dict(consts)
        m["lh"] = np.ascontiguousarray(lh[sl])
        m["subv"] = np.ascontiguousarray(subv[sl])
        m["esrc"] = np.ascontiguousarray(esrc[sl])
        m["edst"] = np.ascontiguousarray(edst[sl])
        in_maps.append(m)
    flags = (br, bool(np.all(b1 == 0)), bool(np.all(b2 == 0)))
    return in_maps, flags


def _run(inputs, trace=False):
    in_maps, flags = _prepare_in_maps(inputs)
    key = ("prog",) + flags
    if key not in _CACHE:
        _CACHE[key] = build_program(*flags)
    nc = _CACHE[key]
    res = run_bass_kernel_spmd(nc, in_maps, list(range(NCORES)), trace=trace)
    out = np.concatenate([np.asarray(res.results[i]["out"]) for i in range(NCORES)],
                         axis=0).astype(np.float32)
    return out, res


def kernel(**inputs) -> np.ndarray:
    out, _ = _run(inputs, trace=False)
    return out


# revision 14
# speedup vs baseline: 2.5181x; 1.1039x over previous
"""Trainium2 Bass kernel for BioBERT-ARG-GNN (gated pooling + 2-layer GCN + MLP head).

Strategy: pure data parallel over batch B=64 across 8 NeuronCores (8 graphs
per core).  All segment/gather ops are dense matmuls against one-hot
matrices built on-device from the index tensors (N=128 nodes == partition
dim).  GCN normalization (D^-1/2 (A+I) D^-1/2) factors into per-partition
scalings around a dense [128,128] adjacency matmul.  Matmul dtypes: f32r
(TF32-like, 1 cycle/row at free-dim>=256) for the big subtoken pooling,
bf16 for the [128,128] GCN matmuls (adjacency counts are exact), f32 for
the tiny FC head.  Phase split keeps each ACT function's table loaded once.
"""

import os
import sys

import numpy as np

for _p in ("/opt/trn_rl_repo", "/root/.axon_site/_ro/trn_rl_repo"):
    if os.path.isdir(_p) and _p not in sys.path:
        sys.path.insert(0, _p)

import ml_dtypes  # noqa: E402
import concourse.bass as bass  # noqa: E402
import concourse.mybir as mybir  # noqa: E402
from concourse import tile  # noqa: E402
from concourse.bass_utils import run_bass_kernel_spmd  # noqa: E402

# Problem shapes (hardcoded per contest rules).
B, S, H = 64, 512, 768
N, E = 128, 1024
GH, FH, L = 128, 256, 2
NCORES = 8
BL = B // NCORES  # graphs per core
SC = S // 128     # subtoken chunks per graph
EC = E // 128     # edge chunks per graph
HC = H // 128     # BERT-hidden chunks
FC = (H + GH) // 128  # concat-feature chunks for the FC head

f32 = mybir.dt.float32
f32r = mybir.dt.float32r
bf16 = mybir.dt.bfloat16
AFT = mybir.ActivationFunctionType
ALU = mybir.AluOpType
BF16 = ml_dtypes.bfloat16

_CACHE = {}


def _split_multi_waits(nc: bass.Bass) -> int:
    """Walrus in this container accepts one sync-wait per instruction; split
    extra waits into single-wait EventSemaphore nops just before it."""
    n_split = 0
    for fn in nc.m.functions:
        for blk in fn.blocks:
            new_instrs = []
            changed = False
            for inst in blk.instructions:
                si = getattr(inst, "sync_info", None)
                if si is not None and si.on_wait is not None and len(si.on_wait) > 1:
                    waits = list(si.on_wait)
                    for j, w in enumerate(waits[:-1]):
                        ev = mybir.InstEventSemaphore(
                            name=f"{inst.name}_ws{j}",
                            ins=[], outs=[],
                            engine=inst.engine,
                            sync_info=mybir.SyncInfo(on_wait=[w], on_update=[]),
                        )
                        new_instrs.append(ev)
                    inst.sync_info = mybir.SyncInfo(
                        on_wait=[waits[-1]], on_update=list(si.on_update))
                    n_split += 1
                    changed = True
                new_instrs.append(inst)
            if changed:
                blk.instructions = new_instrs
    return n_split


def build_program(br_val: float, b1_zero: bool, b2_zero: bool) -> bass.Bass:
    nc = bass.Bass()

    lh_d = nc.declare_dram_parameter("lh", [BL, S, H], f32r, isOutput=False)
    subv_d = nc.declare_dram_parameter("subv", [BL, 128, SC], f32, isOutput=False)
    esrc_d = nc.declare_dram_parameter("esrc", [BL, 128, EC], f32, isOutput=False)
    edst_d = nc.declare_dram_parameter("edst", [BL, 128, EC], f32, isOutput=False)
    wrb_d = nc.declare_dram_parameter("wrb", [128, H], f32, isOutput=False)
    w1t_d = nc.declare_dram_parameter("w1t", [128, HC, GH], bf16, isOutput=False)
    w2t_d = nc.declare_dram_parameter("w2t", [GH, GH], bf16, isOutput=False)
    wf1t_d = nc.declare_dram_parameter("wf1t", [128, FC, FH], f32, isOutput=False)
    wf2t_d = nc.declare_dram_parameter("wf2t", [128, 2, L], f32, isOutput=False)
    b1b_d = nc.declare_dram_parameter("b1b", [128, GH], f32, isOutput=False)
    b2b_d = nc.declare_dram_parameter("b2b", [128, GH], f32, isOutput=False)
    bf1b_d = nc.declare_dram_parameter("bf1b", [BL, FH], f32, isOutput=False)
    bf2b_d = nc.declare_dram_parameter("bf2b", [BL, L], f32, isOutput=False)
    iotaf_d = nc.declare_dram_parameter("iota_f", [128, 128], f32, isOutput=False)
    iota8_d = nc.declare_dram_parameter("iota8", [128, EC, 128], f32, isOutput=False)
    identb_d = nc.declare_dram_parameter("ident_b", [128, 128], bf16, isOutput=False)
    identf_d = nc.declare_dram_parameter("ident_f", [128, 128], f32, isOutput=False)
    onesr_d = nc.declare_dram_parameter("ones_r", [128, 1], f32r, isOutput=False)
    onesb_d = nc.declare_dram_parameter("ones_b", [128, 1], bf16, isOutput=False)
    meanb_d = nc.declare_dram_parameter("mean_b", [128, 1], bf16, isOutput=False)
    out_d = nc.declare_dram_parameter("out", [BL, L], f32, isOutput=True)

    with tile.TileContext(nc) as tc:
        with (
            tc.tile_pool(name="const", bufs=1) as cpool,
            tc.tile_pool(name="lhp", bufs=6) as lhpool,
            tc.tile_pool(name="scr", bufs=2) as scpool,
            tc.tile_pool(name="work", bufs=3) as wpool,
            tc.tile_pool(name="small", bufs=6) as spool,
            tc.tile_pool(name="psA", bufs=2, space="PSUM") as psA,
            tc.tile_pool(name="psB", bufs=2, space="PSUM") as psB,
            tc.tile_pool(name="psC", bufs=2, space="PSUM") as psC,
        ):
            # ---- constants (loaded once) ----
            iota_f = cpool.tile([128, 128], f32)
            nc.sync.dma_start(iota_f[:], iotaf_d[:])
            iota8 = cpool.tile([128, EC, 128], f32)
            nc.sync.dma_start(iota8[:], iota8_d[:])
            ident_b = cpool.tile([128, 128], bf16)
            nc.sync.dma_start(ident_b[:], identb_d[:])
            ident_f = cpool.tile([128, 128], f32)
            nc.sync.dma_start(ident_f[:], identf_d[:])
            wrb = cpool.tile([128, H], f32)
            nc.sync.dma_start(wrb[:], wrb_d[:])
            w1s = cpool.tile([128, HC, GH], bf16)
            nc.sync.dma_start(w1s[:], w1t_d[:])
            w2s = cpool.tile([GH, GH], bf16)
            nc.sync.dma_start(w2s[:], w2t_d[:])
            wf1s = cpool.tile([128, FC, FH], f32)
            nc.sync.dma_start(wf1s[:], wf1t_d[:])
            wf2s = cpool.tile([128, 2, L], f32)
            nc.sync.dma_start(wf2s[:], wf2t_d[:])
            b1b = cpool.tile([128, GH], f32)
            nc.sync.dma_start(b1b[:], b1b_d[:])
            b2b = cpool.tile([128, GH], f32)
            nc.sync.dma_start(b2b[:], b2b_d[:])
            bf1b = cpool.tile([BL, FH], f32)
            nc.sync.dma_start(bf1b[:], bf1b_d[:])
            bf2b = cpool.tile([BL, L], f32)
            nc.sync.dma_start(bf2b[:], bf2b_d[:])
            ones_r = cpool.tile([128, 1], f32r)
            nc.sync.dma_start(ones_r[:], onesr_d[:])
            ones_b = cpool.tile([128, 1], bf16)
            nc.sync.dma_start(ones_b[:], onesb_d[:])
            mean_b = cpool.tile([128, 1], bf16)
            nc.sync.dma_start(mean_b[:], meanb_d[:])
            # pooled graph embeddings (written one column per graph)
            catT6 = cpool.tile([128, BL], f32)

            # ---------- phase 0: adjacency + degrees for all graphs ----------
            atis = []
            dinvs = []
            subvs = []
            for g in range(BL):
                subv = spool.tile([128, SC], f32, tag="subv", bufs=BL)
                nc.sync.dma_start(subv[:], subv_d[g])
                subvs.append(subv)
                esrc = spool.tile([128, EC], f32, tag="esrc", bufs=2)
                nc.sync.dma_start(esrc[:], esrc_d[g])
                edst = spool.tile([128, EC], f32, tag="edst", bufs=2)
                nc.sync.dma_start(edst[:], edst_d[g])

                at_ps = psB.tile([128, 128], f32, tag="mm")
                s_all = wpool.tile([128, EC, 128], bf16, tag="ohS")
                nc.vector.tensor_tensor(
                    out=s_all[:], in0=esrc[:].broadcast_to([128, EC, 128]),
                    in1=iota8[:], op=ALU.is_equal)
                d_all = wpool.tile([128, EC, 128], bf16, tag="ohD")
                nc.vector.tensor_tensor(
                    out=d_all[:], in0=edst[:].broadcast_to([128, EC, 128]),
                    in1=iota8[:], op=ALU.is_equal)
                for c in range(EC):
                    nc.tensor.matmul(at_ps[:], s_all[:, c, :], d_all[:, c, :],
                                     start=(c == 0), stop=False)
                # += I (self-loops) via identity outer product, exact in bf16
                nc.tensor.matmul(at_ps[:], ident_b[:], ident_b[:], start=False,
                                 stop=True)
                ati = wpool.tile([128, 128], bf16, tag="ati", bufs=BL)
                nc.scalar.copy(ati[:], at_ps[:])
                atis.append(ati)
                # deg[d] = sum_s ATI[s,d]  -> dinv = 1/sqrt(deg)
                deg_ps = psB.tile([128, 1], f32, tag="mm")
                nc.tensor.matmul(deg_ps[:], ati[:], ones_b[:],
                                 start=True, stop=True)
                sdeg = spool.tile([128, 1], f32, tag="sv")
                nc.scalar.activation(sdeg[:], deg_ps[:], AFT.Sqrt)
                dinv = spool.tile([128, 1], f32, tag="dinv", bufs=BL)
                nc.vector.reciprocal(dinv[:], sdeg[:])
                dinvs.append(dinv)

            # ---------- phase 1: gate + pooling + GCN per graph ----------
            for g in range(BL):
                subv = subvs[g]
                ati = atis[g]
                dinv = dinvs[g]

                cnt_ps = psC.tile([128, SC], f32, tag="cnt")
                nf_ps = psA.tile([128, H], f32, tag="nf")
                p_all = wpool.tile([128, SC, 128], bf16, tag="ohP")
                nc.vector.tensor_tensor(
                    out=p_all[:], in0=subv[:].broadcast_to([128, SC, 128]),
                    in1=iota8[:, 0:SC, :], op=ALU.is_equal)
                for c in range(SC):
                    lht = lhpool.tile([128, H], f32r, tag="lh")
                    nc.sync.dma_start(lht[:], lh_d[g, c * 128 : (c + 1) * 128, :])
                    scr = scpool.tile([128, H], bf16, tag="scr")
                    logits = spool.tile([128, 1], f32, tag="sv")
                    nc.vector.scalar_tensor_tensor(
                        scr[:], lht[:].bitcast(f32), 0.0, wrb[:], ALU.bypass,
                        ALU.mult, accum_out=logits[:])
                    gate = spool.tile([128, 1], f32, tag="sv")
                    nc.scalar.activation(gate[:], logits[:], AFT.Sigmoid,
                                         bias=float(br_val))
                    pg_t = wpool.tile([128, 128], f32r, tag="ohPg")
                    nc.scalar.mul(pg_t[:], p_all[:, c, :], gate[:])
                    nc.tensor.matmul(cnt_ps[:, c : c + 1], p_all[:, c, :],
                                     ones_b[:], start=True, stop=True)
                    # pooled node feats: nf[n,h] += Pg[s,n]^T lh[s,h]
                    nc.tensor.matmul(nf_ps[:, 0:512], pg_t[:], lht[:, 0:512],
                                     start=(c == 0), stop=(c == SC - 1))
                    nc.tensor.matmul(nf_ps[:, 512:H], pg_t[:], lht[:, 512:H],
                                     start=(c == 0), stop=(c == SC - 1))

                # 1/max(cnt,1); combined layer-1 row scale s1 = invc * dinv
                cnt1 = spool.tile([128, 1], f32, tag="sv")
                nc.vector.tensor_reduce(cnt1[:], cnt_ps[:], mybir.AxisListType.X,
                                        ALU.add)
                mx = spool.tile([128, 1], f32, tag="sv")
                nc.vector.tensor_scalar_max(mx[:], cnt1[:], 1.0)
                invc = spool.tile([128, 1], f32, tag="sv")
                nc.vector.reciprocal(invc[:], mx[:])
                s1 = spool.tile([128, 1], f32, tag="sv")
                nc.vector.tensor_tensor(s1[:], invc[:], dinv[:], ALU.mult)

                # scale rows by s1 while moving PSUM->SBUF (bf16 for layer 1)
                nf_sb = wpool.tile([128, H], bf16, tag="nfsb", bufs=2)
                nc.vector.tensor_scalar_mul(nf_sb[:], nf_ps[:], s1[:])
                # transpose to nfT chunks [h,n]
                nfs = wpool.tile([128, HC, GH], bf16, tag="nfs", bufs=2)
                for hc in range(HC):
                    tr_ps = psB.tile([128, 128], bf16, tag="mm")
                    nc.tensor.transpose(tr_ps[:], nf_sb[:, hc * 128 : (hc + 1) * 128],
                                        ident_b[:])
                    nc.any.tensor_copy(nfs[:, hc, :], tr_ps[:])

                # GCN layer 1: T2 = (s1*sums) @ W1  (scale pre-applied)
                t1_ps = psB.tile([128, GH], f32, tag="mm")
                for hc in range(HC):
                    nc.tensor.matmul(t1_ps[:], nfs[:, hc, :], w1s[:, hc, :],
                                     start=(hc == 0), stop=(hc == HC - 1))
                t2 = wpool.tile([128, GH], bf16, tag="t2")
                nc.any.tensor_copy(t2[:], t1_ps[:])
                z_ps = psB.tile([128, GH], f32, tag="mm")
                nc.tensor.matmul(z_ps[:], ati[:], t2[:], start=True, stop=True)
                x1 = wpool.tile([128, GH], bf16, tag="x1")
                if b1_zero:
                    # x1 = dinv * relu(z)  (valid since dinv > 0)
                    nc.vector.tensor_scalar(x1[:], z_ps[:], 0.0, dinv[:],
                                            ALU.max, ALU.mult)
                else:
                    x1p = wpool.tile([128, GH], f32, tag="x1p")
                    nc.vector.scalar_tensor_tensor(x1p[:], z_ps[:], dinv[:],
                                                   b1b[:], ALU.mult, ALU.add)
                    nc.vector.tensor_scalar_max(x1[:], x1p[:], 0.0)

                # GCN layer 2
                x1t_ps = psB.tile([128, GH], bf16, tag="mm")
                nc.tensor.transpose(x1t_ps[:], x1[:], ident_b[:])
                x1t = wpool.tile([128, GH], bf16, tag="x1t")
                nc.any.tensor_copy(x1t[:], x1t_ps[:])
                tp_ps = psB.tile([128, GH], f32, tag="mm")
                nc.tensor.matmul(tp_ps[:], x1t[:], w2s[:], start=True, stop=True)
                t2p = wpool.tile([128, GH], bf16, tag="t2")
                nc.vector.tensor_scalar_mul(t2p[:], tp_ps[:], dinv[:])
                z2_ps = psB.tile([128, GH], f32, tag="mm")
                nc.tensor.matmul(z2_ps[:], ati[:], t2p[:], start=True, stop=True)
                x2 = wpool.tile([128, GH], bf16, tag="x1")
                if b2_zero:
                    nc.vector.tensor_scalar(x2[:], z2_ps[:], 0.0, dinv[:],
                                            ALU.max, ALU.mult)
                else:
                    x2p = wpool.tile([128, GH], f32, tag="x1p")
                    nc.vector.scalar_tensor_tensor(x2p[:], z2_ps[:], dinv[:],
                                                   b2b[:], ALU.mult, ALU.add)
                    nc.vector.tensor_scalar_max(x2[:], x2p[:], 0.0)

                # graph mean pool -> column g of catT6
                pool_ps = psB.tile([128, 1], f32, tag="mm")
                nc.tensor.matmul(pool_ps[:], x2[:], mean_b[:], start=True,
                                 stop=True)
                nc.scalar.copy(catT6[:, g : g + 1], pool_ps[:])

            # ---------- FC head over all BL graphs ----------
            clsr = cpool.tile([BL, H], f32)
            nc.sync.dma_start(clsr[:], lh_d[:, 0, :].bitcast(f32))
            h1_ps = psB.tile([BL, FH], f32, tag="mm")
            for c in range(FC):
                if c < HC:
                    ct_ps = psB.tile([128, BL], f32, tag="mm")
                    nc.tensor.transpose(ct_ps[:], clsr[:, c * 128 : (c + 1) * 128],
                                        ident_f[0:BL, 0:BL])
                    catc = wpool.tile([128, BL], f32, tag="catc", bufs=2)
                    nc.any.tensor_copy(catc[:], ct_ps[:])
                else:
                    catc = catT6
                nc.tensor.matmul(h1_ps[:], catc[:], wf1s[:, c, :], start=(c == 0),
                                 stop=(c == FC - 1))
            h1s = wpool.tile([BL, FH], f32, tag="h1")
            nc.vector.scalar_tensor_tensor(h1s[:], h1_ps[:], 1.0, bf1b[:],
                                           ALU.bypass, ALU.add)
            hr = wpool.tile([BL, FH], f32, tag="h1")
            nc.vector.tensor_scalar_max(hr[:], h1s[:], 0.0)
            out_ps = psB.tile([BL, L], f32, tag="mm")
            for c in range(2):
                ht_ps = psB.tile([128, BL], f32, tag="mm")
                nc.tensor.transpose(ht_ps[:], hr[:, c * 128 : (c + 1) * 128],
                                    ident_f[0:BL, 0:BL])
                htc = wpool.tile([128, BL], f32, tag="catc", bufs=2)
                nc.any.tensor_copy(htc[:], ht_ps[:])
                nc.tensor.matmul(out_ps[:], htc[:], wf2s[:, c, :], start=(c == 0),
                                 stop=(c == 1))
            outs = wpool.tile([BL, L], f32, tag="outs")
            nc.vector.scalar_tensor_tensor(outs[:], out_ps[:], 1.0, bf2b[:],
                                           ALU.bypass, ALU.add)
            nc.sync.dma_start(out_d[:], outs[:])

    _split_multi_waits(nc)
    return nc


def _prepare_in_maps(inputs):
    lh = np.ascontiguousarray(np.asarray(inputs["last_hidden"], dtype=np.float32))
    submap = np.asarray(inputs["submap"]).astype(np.int64)
    edge_index = np.asarray(inputs["edge_index"]).astype(np.int64)
    assert lh.shape == (B, S, H)
    assert int(inputs.get("num_nodes", N)) == N

    wr = np.asarray(inputs["wr"], dtype=np.float32)
    br = float(np.asarray(inputs["br"], dtype=np.float32))
    W1 = np.asarray(inputs["W1"], dtype=np.float32)
    b1 = np.asarray(inputs["b1"], dtype=np.float32)
    W2 = np.asarray(inputs["W2"], dtype=np.float32)
    b2 = np.asarray(inputs["b2"], dtype=np.float32)
    Wf1 = np.asarray(inputs["Wf1"], dtype=np.float32)
    bf1 = np.asarray(inputs["bf1"], dtype=np.float32)
    Wf2 = np.asarray(inputs["Wf2"], dtype=np.float32)
    bf2 = np.asarray(inputs["bf2"], dtype=np.float32)

    # Shared (replicated) tensors.
    consts = {
        "wrb": np.ascontiguousarray(np.broadcast_to(wr, (128, H))),
        "w1t": np.ascontiguousarray(
            W1.reshape(HC, 128, GH).transpose(1, 0, 2)).astype(BF16),
        "w2t": np.ascontiguousarray(W2).astype(BF16),
        "wf1t": np.ascontiguousarray(
            Wf1.reshape(FC, 128, FH).transpose(1, 0, 2)),
        "wf2t": np.ascontiguousarray(
            Wf2.reshape(2, 128, L).transpose(1, 0, 2)),
        "b1b": np.ascontiguousarray(np.broadcast_to(b1, (128, GH))),
        "b2b": np.ascontiguousarray(np.broadcast_to(b2, (128, GH))),
        "bf1b": np.ascontiguousarray(np.broadcast_to(bf1, (BL, FH))),
        "bf2b": np.ascontiguousarray(np.broadcast_to(bf2, (BL, L))),
        "iota_f": np.ascontiguousarray(
            np.broadcast_to(np.arange(128, dtype=np.float32), (128, 128))),
        "iota8": np.ascontiguousarray(
            np.broadcast_to(np.arange(128, dtype=np.float32), (128, EC, 128))),
        "ident_b": np.eye(128, dtype=np.float32).astype(BF16),
        "ident_f": np.eye(128, dtype=np.float32),
        "ones_r": np.ones((128, 1), np.float32),
        "ones_b": np.ones((128, 1), np.float32).astype(BF16),
        "mean_b": np.full((128, 1), 1.0 / N, np.float32).astype(BF16),
    }

    # Per-graph index layouts: value of token t goes to partition t%128,
    # column t//128.
    subv = submap.reshape(B, SC, 128).transpose(0, 2, 1).astype(np.float32)
    esrc = edge_index[:, 0, :].reshape(B, EC, 128).transpose(0, 2, 1).astype(np.float32)
    edst = edge_index[:, 1, :].reshape(B, EC, 128).transpose(0, 2, 1).astype(np.float32)

    in_maps = []
    for i in range(NCORES):
        sl = slice(i * BL, (i + 1) * BL)
        m = dict(consts)
        m["lh"] = np.ascontiguousarray(lh[sl])
        m["subv"] = np.ascontiguousarray(subv[sl])
        m["esrc"] = np.ascontiguousarray(esrc[sl])
        m["edst"] = np.ascontiguousarray(edst[sl])
        in_maps.append(m)
    flags = (br, bool(np.all(b1 == 0)), bool(np.all(b2 == 0)))
    return in_maps, flags


def _run(inputs, trace=False):
    in_maps, flags = _prepare_in_maps(inputs)
    key = ("prog",) + flags
    if key not in _CACHE:
        _CACHE[key] = build_program(*flags)
    nc = _CACHE[key]
    res = run_bass_kernel_spmd(nc, in_maps, list(range(NCORES)), trace=trace)
    out = np.concatenate([np.asarray(res.results[i]["out"]) for i in range(NCORES)],
                         axis=0).astype(np.float32)
    return out, res


def kernel(**inputs) -> np.ndarray:
    out, _ = _run(inputs, trace=False)
    return out


# revision 15
# speedup vs baseline: 2.5470x; 1.0115x over previous
"""Trainium2 Bass kernel for BioBERT-ARG-GNN (gated pooling + 2-layer GCN + MLP head).

Strategy: pure data parallel over batch B=64 across 8 NeuronCores (8 graphs
per core).  All segment/gather ops are dense matmuls against one-hot
matrices built on-device from the index tensors (N=128 nodes == partition
dim).  GCN normalization (D^-1/2 (A+I) D^-1/2) factors into per-partition
scalings around a dense [128,128] adjacency matmul.  Matmul dtypes: f32r
(TF32-like, 1 cycle/row at free-dim>=256) for the big subtoken pooling,
bf16 for the [128,128] GCN matmuls (adjacency counts are exact), f32 for
the tiny FC head.  Phase split keeps each ACT function's table loaded once.
"""

import os
import sys

import numpy as np

for _p in ("/opt/trn_rl_repo", "/root/.axon_site/_ro/trn_rl_repo"):
    if os.path.isdir(_p) and _p not in sys.path:
        sys.path.insert(0, _p)

import ml_dtypes  # noqa: E402
import concourse.bass as bass  # noqa: E402
import concourse.mybir as mybir  # noqa: E402
from concourse import tile  # noqa: E402
from concourse.bass_utils import run_bass_kernel_spmd  # noqa: E402

# Problem shapes (hardcoded per contest rules).
B, S, H = 64, 512, 768
N, E = 128, 1024
GH, FH, L = 128, 256, 2
NCORES = 8
BL = B // NCORES  # graphs per core
SC = S // 128     # subtoken chunks per graph
EC = E // 128     # edge chunks per graph
HC = H // 128     # BERT-hidden chunks
FC = (H + GH) // 128  # concat-feature chunks for the FC head

f32 = mybir.dt.float32
f32r = mybir.dt.float32r
bf16 = mybir.dt.bfloat16
AFT = mybir.ActivationFunctionType
ALU = mybir.AluOpType
BF16 = ml_dtypes.bfloat16

_CACHE = {}


def _split_multi_waits(nc: bass.Bass) -> int:
    """Walrus in this container accepts one sync-wait per instruction; split
    extra waits into single-wait EventSemaphore nops just before it."""
    n_split = 0
    for fn in nc.m.functions:
        for blk in fn.blocks:
            new_instrs = []
            changed = False
            for inst in blk.instructions:
                si = getattr(inst, "sync_info", None)
                if si is not None and si.on_wait is not None and len(si.on_wait) > 1:
                    waits = list(si.on_wait)
                    for j, w in enumerate(waits[:-1]):
                        ev = mybir.InstEventSemaphore(
                            name=f"{inst.name}_ws{j}",
                            ins=[], outs=[],
                            engine=inst.engine,
                            sync_info=mybir.SyncInfo(on_wait=[w], on_update=[]),
                        )
                        new_instrs.append(ev)
                    inst.sync_info = mybir.SyncInfo(
                        on_wait=[waits[-1]], on_update=list(si.on_update))
                    n_split += 1
                    changed = True
                new_instrs.append(inst)
            if changed:
                blk.instructions = new_instrs
    return n_split


def build_program(br_val: float, b1_zero: bool, b2_zero: bool) -> bass.Bass:
    nc = bass.Bass()

    lh_d = nc.declare_dram_parameter("lh", [BL, S, H], f32r, isOutput=False)
    subv_d = nc.declare_dram_parameter("subv", [BL, 128, SC], f32, isOutput=False)
    esrc_d = nc.declare_dram_parameter("esrc", [BL, 128, EC], f32, isOutput=False)
    edst_d = nc.declare_dram_parameter("edst", [BL, 128, EC], f32, isOutput=False)
    wrb_d = nc.declare_dram_parameter("wrb", [128, H], f32, isOutput=False)
    w1t_d = nc.declare_dram_parameter("w1t", [128, HC, GH], bf16, isOutput=False)
    w2t_d = nc.declare_dram_parameter("w2t", [GH, GH], bf16, isOutput=False)
    wf1t_d = nc.declare_dram_parameter("wf1t", [128, FC, FH], f32, isOutput=False)
    wf2t_d = nc.declare_dram_parameter("wf2t", [128, 2, L], f32, isOutput=False)
    b1b_d = nc.declare_dram_parameter("b1b", [128, GH], f32, isOutput=False)
    b2b_d = nc.declare_dram_parameter("b2b", [128, GH], f32, isOutput=False)
    bf1b_d = nc.declare_dram_parameter("bf1b", [BL, FH], f32, isOutput=False)
    bf2b_d = nc.declare_dram_parameter("bf2b", [BL, L], f32, isOutput=False)
    iotaf_d = nc.declare_dram_parameter("iota_f", [128, 128], f32, isOutput=False)
    iota8_d = nc.declare_dram_parameter("iota8", [128, EC, 128], f32, isOutput=False)
    identb_d = nc.declare_dram_parameter("ident_b", [128, 128], bf16, isOutput=False)
    identf_d = nc.declare_dram_parameter("ident_f", [128, 128], f32, isOutput=False)
    onesr_d = nc.declare_dram_parameter("ones_r", [128, 1], f32r, isOutput=False)
    onesb_d = nc.declare_dram_parameter("ones_b", [128, 1], bf16, isOutput=False)
    meanb_d = nc.declare_dram_parameter("mean_b", [128, 1], bf16, isOutput=False)
    out_d = nc.declare_dram_parameter("out", [BL, L], f32, isOutput=True)

    with tile.TileContext(nc) as tc:
        with (
            tc.tile_pool(name="const", bufs=1) as cpool,
            tc.tile_pool(name="lhp", bufs=8) as lhpool,
            tc.tile_pool(name="scr", bufs=3) as scpool,
            tc.tile_pool(name="work", bufs=3) as wpool,
            tc.tile_pool(name="small", bufs=6) as spool,
            tc.tile_pool(name="psA", bufs=2, space="PSUM") as psA,
            tc.tile_pool(name="psB", bufs=2, space="PSUM") as psB,
            tc.tile_pool(name="psC", bufs=2, space="PSUM") as psC,
        ):
            # ---- early constants (ACT HWDGE ring; SP ring is reserved for lh) ----
            iota8 = cpool.tile([128, EC, 128], f32)
            nc.scalar.dma_start(iota8[:], iota8_d[:])
            ident_b = cpool.tile([128, 128], bf16)
            nc.scalar.dma_start(ident_b[:], identb_d[:])
            ones_b = cpool.tile([128, 1], bf16)
            nc.scalar.dma_start(ones_b[:], onesb_d[:])
            wrb = cpool.tile([128, H], f32)
            nc.scalar.dma_start(wrb[:], wrb_d[:])
            w1s = cpool.tile([128, HC, GH], bf16)
            nc.scalar.dma_start(w1s[:], w1t_d[:])
            w2s = cpool.tile([GH, GH], bf16)
            nc.scalar.dma_start(w2s[:], w2t_d[:])
            mean_b = cpool.tile([128, 1], bf16)
            nc.scalar.dma_start(mean_b[:], meanb_d[:])
            # pooled graph embeddings (written one column per graph)
            catT6 = cpool.tile([128, BL], f32)

            # ---------- phase 0: adjacency + degrees for all graphs ----------
            atis = []
            dinvs = []
            subvs = []
            for g in range(BL):
                subv = spool.tile([128, SC], f32, tag="subv", bufs=BL)
                nc.scalar.dma_start(subv[:], subv_d[g])
                subvs.append(subv)
                esrc = spool.tile([128, EC], f32, tag="esrc", bufs=2)
                nc.scalar.dma_start(esrc[:], esrc_d[g])
                edst = spool.tile([128, EC], f32, tag="edst", bufs=2)
                nc.scalar.dma_start(edst[:], edst_d[g])

                at_ps = psB.tile([128, 128], f32, tag="mm")
                s_all = wpool.tile([128, EC, 128], bf16, tag="ohS")
                nc.vector.tensor_tensor(
                    out=s_all[:], in0=esrc[:].broadcast_to([128, EC, 128]),
                    in1=iota8[:], op=ALU.is_equal)
                d_all = wpool.tile([128, EC, 128], bf16, tag="ohD")
                nc.vector.tensor_tensor(
                    out=d_all[:], in0=edst[:].broadcast_to([128, EC, 128]),
                    in1=iota8[:], op=ALU.is_equal)
                for c in range(EC):
                    nc.tensor.matmul(at_ps[:], s_all[:, c, :], d_all[:, c, :],
                                     start=(c == 0), stop=False)
                # += I (self-loops) via identity outer product, exact in bf16
                nc.tensor.matmul(at_ps[:], ident_b[:], ident_b[:], start=False,
                                 stop=True)
                ati = wpool.tile([128, 128], bf16, tag="ati", bufs=BL)
                nc.scalar.copy(ati[:], at_ps[:])
                atis.append(ati)
                # deg[d] = sum_s ATI[s,d]  -> dinv = 1/sqrt(deg)
                deg_ps = psB.tile([128, 1], f32, tag="mm")
                nc.tensor.matmul(deg_ps[:], ati[:], ones_b[:],
                                 start=True, stop=True)
                sdeg = spool.tile([128, 1], f32, tag="sv")
                nc.scalar.activation(sdeg[:], deg_ps[:], AFT.Sqrt)
                dinv = spool.tile([128, 1], f32, tag="dinv", bufs=BL)
                nc.vector.reciprocal(dinv[:], sdeg[:])
                dinvs.append(dinv)

            # ---------- phase 1: gate + pooling + GCN per graph ----------
            _b1b = [None]
            _b2b = [None]
            for g in range(BL):
                subv = subvs[g]
                ati = atis[g]
                dinv = dinvs[g]

                cnt_ps = psC.tile([128, SC], f32, tag="cnt")
                nf_ps = psA.tile([128, H], f32, tag="nf")
                p_all = wpool.tile([128, SC, 128], bf16, tag="ohP")
                nc.vector.tensor_tensor(
                    out=p_all[:], in0=subv[:].broadcast_to([128, SC, 128]),
                    in1=iota8[:, 0:SC, :], op=ALU.is_equal)
                for c in range(SC):
                    lht = lhpool.tile([128, H], f32r, tag="lh")
                    nc.sync.dma_start(lht[:], lh_d[g, c * 128 : (c + 1) * 128, :])
                    scr = scpool.tile([128, H], bf16, tag="scr")
                    logits = spool.tile([128, 1], f32, tag="sv")
                    nc.vector.scalar_tensor_tensor(
                        scr[:], lht[:].bitcast(f32), 0.0, wrb[:], ALU.bypass,
                        ALU.mult, accum_out=logits[:])
                    gate = spool.tile([128, 1], f32, tag="sv")
                    nc.scalar.activation(gate[:], logits[:], AFT.Sigmoid,
                                         bias=float(br_val))
                    pg_t = wpool.tile([128, 128], f32r, tag="ohPg")
                    nc.scalar.mul(pg_t[:], p_all[:, c, :], gate[:])
                    nc.tensor.matmul(cnt_ps[:, c : c + 1], p_all[:, c, :],
                                     ones_b[:], start=True, stop=True)
                    # pooled node feats: nf[n,h] += Pg[s,n]^T lh[s,h]
                    nc.tensor.matmul(nf_ps[:, 0:512], pg_t[:], lht[:, 0:512],
                                     start=(c == 0), stop=(c == SC - 1))
                    nc.tensor.matmul(nf_ps[:, 512:H], pg_t[:], lht[:, 512:H],
                                     start=(c == 0), stop=(c == SC - 1))

                # 1/max(cnt,1); combined layer-1 row scale s1 = invc * dinv
                cnt1 = spool.tile([128, 1], f32, tag="sv")
                nc.vector.tensor_reduce(cnt1[:], cnt_ps[:], mybir.AxisListType.X,
                                        ALU.add)
                mx = spool.tile([128, 1], f32, tag="sv")
                nc.vector.tensor_scalar_max(mx[:], cnt1[:], 1.0)
                invc = spool.tile([128, 1], f32, tag="sv")
                nc.vector.reciprocal(invc[:], mx[:])
                s1 = spool.tile([128, 1], f32, tag="sv")
                nc.vector.tensor_tensor(s1[:], invc[:], dinv[:], ALU.mult)

                # scale rows by s1 while moving PSUM->SBUF (bf16 for layer 1)
                nf_sb = wpool.tile([128, H], bf16, tag="nfsb", bufs=2)
                nc.vector.tensor_scalar_mul(nf_sb[:], nf_ps[:], s1[:])
                # transpose to nfT chunks [h,n]
                nfs = wpool.tile([128, HC, GH], bf16, tag="nfs", bufs=2)
                for hc in range(HC):
                    tr_ps = psB.tile([128, 128], bf16, tag="mm")
                    nc.tensor.transpose(tr_ps[:], nf_sb[:, hc * 128 : (hc + 1) * 128],
                                        ident_b[:])
                    nc.any.tensor_copy(nfs[:, hc, :], tr_ps[:])

                # GCN layer 1: T2 = (s1*sums) @ W1  (scale pre-applied)
                t1_ps = psB.tile([128, GH], f32, tag="mm")
                for hc in range(HC):
                    nc.tensor.matmul(t1_ps[:], nfs[:, hc, :], w1s[:, hc, :],
                                     start=(hc == 0), stop=(hc == HC - 1))
                t2 = wpool.tile([128, GH], bf16, tag="t2")
                nc.any.tensor_copy(t2[:], t1_ps[:])
                z_ps = psB.tile([128, GH], f32, tag="mm")
                nc.tensor.matmul(z_ps[:], ati[:], t2[:], start=True, stop=True)
                x1 = wpool.tile([128, GH], bf16, tag="x1")
                if b1_zero:
                    # x1 = dinv * relu(z)  (valid since dinv > 0)
                    nc.vector.tensor_scalar(x1[:], z_ps[:], 0.0, dinv[:],
                                            ALU.max, ALU.mult)
                else:
                    if g == 0 and _b1b[0] is None:
                        _b1b[0] = cpool.tile([128, GH], f32, name="b1bt")
                        nc.scalar.dma_start(_b1b[0][:], b1b_d[:])
                    x1p = wpool.tile([128, GH], f32, tag="x1p")
                    nc.vector.scalar_tensor_tensor(x1p[:], z_ps[:], dinv[:],
                                                   _b1b[0][:], ALU.mult, ALU.add)
                    nc.vector.tensor_scalar_max(x1[:], x1p[:], 0.0)

                # GCN layer 2
                x1t_ps = psB.tile([128, GH], bf16, tag="mm")
                nc.tensor.transpose(x1t_ps[:], x1[:], ident_b[:])
                x1t = wpool.tile([128, GH], bf16, tag="x1t")
                nc.any.tensor_copy(x1t[:], x1t_ps[:])
                tp_ps = psB.tile([128, GH], f32, tag="mm")
                nc.tensor.matmul(tp_ps[:], x1t[:], w2s[:], start=True, stop=True)
                t2p = wpool.tile([128, GH], bf16, tag="t2")
                nc.vector.tensor_scalar_mul(t2p[:], tp_ps[:], dinv[:])
                z2_ps = psB.tile([128, GH], f32, tag="mm")
                nc.tensor.matmul(z2_ps[:], ati[:], t2p[:], start=True, stop=True)
                x2 = wpool.tile([128, GH], bf16, tag="x1")
                if b2_zero:
                    nc.vector.tensor_scalar(x2[:], z2_ps[:], 0.0, dinv[:],
                                            ALU.max, ALU.mult)
                else:
                    if g == 0 and _b2b[0] is None:
                        _b2b[0] = cpool.tile([128, GH], f32, name="b2bt")
                        nc.scalar.dma_start(_b2b[0][:], b2b_d[:])
                    x2p = wpool.tile([128, GH], f32, tag="x1p")
                    nc.vector.scalar_tensor_tensor(x2p[:], z2_ps[:], dinv[:],
                                                   _b2b[0][:], ALU.mult, ALU.add)
                    nc.vector.tensor_scalar_max(x2[:], x2p[:], 0.0)

                # graph mean pool -> column g of catT6
                pool_ps = psB.tile([128, 1], f32, tag="mm")
                nc.tensor.matmul(pool_ps[:], x2[:], mean_b[:], start=True,
                                 stop=True)
                nc.scalar.copy(catT6[:, g : g + 1], pool_ps[:])

            # ---------- FC head over all BL graphs ----------
            ident_f = cpool.tile([128, 128], f32)
            nc.scalar.dma_start(ident_f[:], identf_d[:])
            wf1s = cpool.tile([128, FC, FH], f32)
            nc.scalar.dma_start(wf1s[:], wf1t_d[:])
            wf2s = cpool.tile([128, 2, L], f32)
            nc.scalar.dma_start(wf2s[:], wf2t_d[:])
            bf1b = cpool.tile([BL, FH], f32)
            nc.scalar.dma_start(bf1b[:], bf1b_d[:])
            bf2b = cpool.tile([BL, L], f32)
            nc.scalar.dma_start(bf2b[:], bf2b_d[:])
            clsr = cpool.tile([BL, H], f32)
            nc.sync.dma_start(clsr[:], lh_d[:, 0, :].bitcast(f32))
            h1_ps = psB.tile([BL, FH], f32, tag="mm")
            for c in range(FC):
                if c < HC:
                    ct_ps = psB.tile([128, BL], f32, tag="mm")
                    nc.tensor.transpose(ct_ps[:], clsr[:, c * 128 : (c + 1) * 128],
                                        ident_f[0:BL, 0:BL])
                    catc = wpool.tile([128, BL], f32, tag="catc", bufs=2)
                    nc.any.tensor_copy(catc[:], ct_ps[:])
                else:
                    catc = catT6
                nc.tensor.matmul(h1_ps[:], catc[:], wf1s[:, c, :], start=(c == 0),
                                 stop=(c == FC - 1))
            h1s = wpool.tile([BL, FH], f32, tag="h1")
            nc.vector.scalar_tensor_tensor(h1s[:], h1_ps[:], 1.0, bf1b[:],
                                           ALU.bypass, ALU.add)
            hr = wpool.tile([BL, FH], f32, tag="h1")
            nc.vector.tensor_scalar_max(hr[:], h1s[:], 0.0)
            out_ps = psB.tile([BL, L], f32, tag="mm")
            for c in range(2):
                ht_ps = psB.tile([128, BL], f32, tag="mm")
                nc.tensor.transpose(ht_ps[:], hr[:, c * 128 : (c + 1) * 128],
                                    ident_f[0:BL, 0:BL])
                htc = wpool.tile([128, BL], f32, tag="catc", bufs=2)
                nc.any.tensor_copy(htc[:], ht_ps[:])
                nc.tensor.matmul(out_ps[:], htc[:], wf2s[:, c, :], start=(c == 0),
                                 stop=(c == 1))
            outs = wpool.tile([BL, L], f32, tag="outs")
            nc.vector.scalar_tensor_tensor(outs[:], out_ps[:], 1.0, bf2b[:],
                                           ALU.bypass, ALU.add)
            nc.sync.dma_start(out_d[:], outs[:])

    _split_multi_waits(nc)
    return nc


def _prepare_in_maps(inputs):
    lh = np.ascontiguousarray(np.asarray(inputs["last_hidden"], dtype=np.float32))
    submap = np.asarray(inputs["submap"]).astype(np.int64)
    edge_index = np.asarray(inputs["edge_index"]).astype(np.int64)
    assert lh.shape == (B, S, H)
    assert int(inputs.get("num_nodes", N)) == N

    wr = np.asarray(inputs["wr"], dtype=np.float32)
    br = float(np.asarray(inputs["br"], dtype=np.float32))
    W1 = np.asarray(inputs["W1"], dtype=np.float32)
    b1 = np.asarray(inputs["b1"], dtype=np.float32)
    W2 = np.asarray(inputs["W2"], dtype=np.float32)
    b2 = np.asarray(inputs["b2"], dtype=np.float32)
    Wf1 = np.asarray(inputs["Wf1"], dtype=np.float32)
    bf1 = np.asarray(inputs["bf1"], dtype=np.float32)
    Wf2 = np.asarray(inputs["Wf2"], dtype=np.float32)
    bf2 = np.asarray(inputs["bf2"], dtype=np.float32)

    # Shared (replicated) tensors.
    consts = {
        "wrb": np.ascontiguousarray(np.broadcast_to(wr, (128, H))),
        "w1t": np.ascontiguousarray(
            W1.reshape(HC, 128, GH).transpose(1, 0, 2)).astype(BF16),
        "w2t": np.ascontiguousarray(W2).astype(BF16),
        "wf1t": np.ascontiguousarray(
            Wf1.reshape(FC, 128, FH).transpose(1, 0, 2)),
        "wf2t": np.ascontiguousarray(
            Wf2.reshape(2, 128, L).transpose(1, 0, 2)),
        "b1b": np.ascontiguousarray(np.broadcast_to(b1, (128, GH))),
        "b2b": np.ascontiguousarray(np.broadcast_to(b2, (128, GH))),
        "bf1b": np.ascontiguousarray(np.broadcast_to(bf1, (BL, FH))),
        "bf2b": np.ascontiguousarray(np.broadcast_to(bf2, (BL, L))),
        "iota_f": np.ascontiguousarray(
            np.broadcast_to(np.arange(128, dtype=np.float32), (128, 128))),
        "iota8": np.ascontiguousarray(
            np.broadcast_to(np.arange(128, dtype=np.float32), (128, EC, 128))),
        "ident_b": np.eye(128, dtype=np.float32).astype(BF16),
        "ident_f": np.eye(128, dtype=np.float32),
        "ones_r": np.ones((128, 1), np.float32),
        "ones_b": np.ones((128, 1), np.float32).astype(BF16),
        "mean_b": np.full((128, 1), 1.0 / N, np.float32).astype(BF16),
    }

    # Per-graph index layouts: value of token t goes to partition t%128,
    # column t//128.
    subv = submap.reshape(B, SC, 128).transpose(0, 2, 1).astype(np.float32)
    esrc = edge_index[:, 0, :].reshape(B, EC, 128).transpose(0, 2, 1).astype(np.float32)
    edst = edge_index[:, 1, :].reshape(B, EC, 128).transpose(0, 2, 1).astype(np.float32)

    in_maps = []
    for i in range(NCORES):
        sl = slice(i * BL, (i + 1) * BL)
        m = dict(consts)
        m["lh"] = np.ascontiguousarray(lh[sl])
        m["subv"] = np.ascontiguousarray(subv[sl])
        m["esrc"] = np.ascontiguousarray(esrc[sl])
        m["edst"] = np.ascontiguousarray(edst[sl])
        in_maps.append(m)
    flags = (br, bool(np.all(b1 == 0)), bool(np.all(b2 == 0)))
    return in_maps, flags


def _run(inputs, trace=False):
    in_maps, flags = _prepare_in_maps(inputs)
    key = ("prog",) + flags
    if key not in _CACHE:
        _CACHE[key] = build_program(*flags)
    nc = _CACHE[key]
    res = run_bass_kernel_spmd(nc, in_maps, list(range(NCORES)), trace=trace)
    out = np.concatenate([np.asarray(res.results[i]["out"]) for i in range(NCORES)],
                         axis=0).astype(np.float32)
    return out, res


def kernel(**inputs) -> np.ndarray:
    out, _ = _run(inputs, trace=False)
    return out


# revision 16
# speedup vs baseline: 2.7630x; 1.0848x over previous
"""Trainium2 Bass kernel for BioBERT-ARG-GNN (gated pooling + 2-layer GCN + MLP head).

Strategy: pure data parallel over batch B=64 across 8 NeuronCores (8 graphs
per core).  All segment/gather ops are dense matmuls against one-hot
matrices built on-device from the index tensors (N=128 nodes == partition
dim).  GCN normalization (D^-1/2 (A+I) D^-1/2) factors into per-partition
scalings around a dense [128,128] adjacency matmul.  Matmul dtypes: f32r
(TF32-like, 1 cycle/row at free-dim>=256) for the big subtoken pooling,
bf16 for the [128,128] GCN matmuls (adjacency counts are exact), f32 for
the tiny FC head.  Phase split keeps each ACT function's table loaded once.
"""

import os
import sys

import numpy as np

for _p in ("/opt/trn_rl_repo", "/root/.axon_site/_ro/trn_rl_repo"):
    if os.path.isdir(_p) and _p not in sys.path:
        sys.path.insert(0, _p)

import ml_dtypes  # noqa: E402
import concourse.bass as bass  # noqa: E402
import concourse.mybir as mybir  # noqa: E402
from concourse import tile  # noqa: E402
from concourse.bass_utils import run_bass_kernel_spmd  # noqa: E402

# Problem shapes (hardcoded per contest rules).
B, S, H = 64, 512, 768
N, E = 128, 1024
GH, FH, L = 128, 256, 2
NCORES = 8
BL = B // NCORES  # graphs per core
SC = S // 128     # subtoken chunks per graph
EC = E // 128     # edge chunks per graph
HC = H // 128     # BERT-hidden chunks
FC = (H + GH) // 128  # concat-feature chunks for the FC head

f32 = mybir.dt.float32
f32r = mybir.dt.float32r
bf16 = mybir.dt.bfloat16
AFT = mybir.ActivationFunctionType
ALU = mybir.AluOpType
BF16 = ml_dtypes.bfloat16

_CACHE = {}


def _split_multi_waits(nc: bass.Bass) -> int:
    """Walrus in this container accepts one sync-wait per instruction; split
    extra waits into single-wait EventSemaphore nops just before it."""
    n_split = 0
    for fn in nc.m.functions:
        for blk in fn.blocks:
            new_instrs = []
            changed = False
            for inst in blk.instructions:
                si = getattr(inst, "sync_info", None)
                if si is not None and si.on_wait is not None and len(si.on_wait) > 1:
                    waits = list(si.on_wait)
                    for j, w in enumerate(waits[:-1]):
                        ev = mybir.InstEventSemaphore(
                            name=f"{inst.name}_ws{j}",
                            ins=[], outs=[],
                            engine=inst.engine,
                            sync_info=mybir.SyncInfo(on_wait=[w], on_update=[]),
                        )
                        new_instrs.append(ev)
                    inst.sync_info = mybir.SyncInfo(
                        on_wait=[waits[-1]], on_update=list(si.on_update))
                    n_split += 1
                    changed = True
                new_instrs.append(inst)
            if changed:
                blk.instructions = new_instrs
    return n_split


def build_program(br_val: float, b1_zero: bool, b2_zero: bool) -> bass.Bass:
    nc = bass.Bass()

    lh_d = nc.declare_dram_parameter("lh", [BL, S, H], f32r, isOutput=False)
    subv_d = nc.declare_dram_parameter("subv", [BL, 128, SC], f32, isOutput=False)
    esrc_d = nc.declare_dram_parameter("esrc", [BL, 128, EC], f32, isOutput=False)
    edst_d = nc.declare_dram_parameter("edst", [BL, 128, EC], f32, isOutput=False)
    wrb_d = nc.declare_dram_parameter("wrb", [128, H], f32, isOutput=False)
    w1t_d = nc.declare_dram_parameter("w1t", [128, HC, GH], bf16, isOutput=False)
    w2t_d = nc.declare_dram_parameter("w2t", [GH, GH], bf16, isOutput=False)
    wf1t_d = nc.declare_dram_parameter("wf1t", [128, FC, FH], f32, isOutput=False)
    wf2t_d = nc.declare_dram_parameter("wf2t", [128, 2, L], f32, isOutput=False)
    b1b_d = nc.declare_dram_parameter("b1b", [128, GH], f32, isOutput=False)
    b2b_d = nc.declare_dram_parameter("b2b", [128, GH], f32, isOutput=False)
    bf1b_d = nc.declare_dram_parameter("bf1b", [BL, FH], f32, isOutput=False)
    bf2b_d = nc.declare_dram_parameter("bf2b", [BL, L], f32, isOutput=False)
    iotaf_d = nc.declare_dram_parameter("iota_f", [128, 128], f32, isOutput=False)
    iota8_d = nc.declare_dram_parameter("iota8", [128, EC, 128], f32, isOutput=False)
    identb_d = nc.declare_dram_parameter("ident_b", [128, 128], bf16, isOutput=False)
    identf_d = nc.declare_dram_parameter("ident_f", [128, 128], f32, isOutput=False)
    onesr_d = nc.declare_dram_parameter("ones_r", [128, 1], f32r, isOutput=False)
    onesb_d = nc.declare_dram_parameter("ones_b", [128, 1], bf16, isOutput=False)
    meanb_d = nc.declare_dram_parameter("mean_b", [128, 1], bf16, isOutput=False)
    out_d = nc.declare_dram_parameter("out", [BL, L], f32, isOutput=True)

    with tile.TileContext(nc) as tc:
        with (
            tc.tile_pool(name="const", bufs=1) as cpool,
            tc.tile_pool(name="lhp", bufs=8) as lhpool,
            tc.tile_pool(name="scr", bufs=3) as scpool,
            tc.tile_pool(name="work", bufs=3) as wpool,
            tc.tile_pool(name="small", bufs=6) as spool,
            tc.tile_pool(name="psA", bufs=2, space="PSUM") as psA,
            tc.tile_pool(name="psB", bufs=2, space="PSUM") as psB,
            tc.tile_pool(name="psC", bufs=2, space="PSUM") as psC,
        ):
            # ---- early constants (ACT HWDGE ring; SP ring is reserved for lh) ----
            iota8 = cpool.tile([128, EC, 128], f32)
            nc.scalar.dma_start(iota8[:], iota8_d[:])
            ident_b = cpool.tile([128, 128], bf16)
            nc.scalar.dma_start(ident_b[:], identb_d[:])
            ones_b = cpool.tile([128, 1], bf16)
            nc.scalar.dma_start(ones_b[:], onesb_d[:])
            wrb = cpool.tile([128, H], f32)
            nc.scalar.dma_start(wrb[:], wrb_d[:])
            w1s = cpool.tile([128, HC, GH], bf16)
            nc.scalar.dma_start(w1s[:], w1t_d[:])
            w2s = cpool.tile([GH, GH], bf16)
            nc.scalar.dma_start(w2s[:], w2t_d[:])
            mean_b = cpool.tile([128, 1], bf16)
            nc.scalar.dma_start(mean_b[:], meanb_d[:])
            # pooled graph embeddings (written one column per graph)
            catT6 = cpool.tile([128, BL], f32)

            # ---------- phase 0: adjacency + degrees for all graphs ----------
            atis = []
            dinvs = []
            subvs = []
            for g in range(BL):
                subv = spool.tile([128, SC], f32, tag="subv", bufs=BL)
                nc.sync.dma_start(subv[:], subv_d[g])
                subvs.append(subv)
                esrc = spool.tile([128, EC], f32, tag="esrc", bufs=2)
                nc.sync.dma_start(esrc[:], esrc_d[g])
                edst = spool.tile([128, EC], f32, tag="edst", bufs=2)
                nc.sync.dma_start(edst[:], edst_d[g])

                at_ps = psB.tile([128, 128], f32, tag="mm")
                s_all = wpool.tile([128, EC, 128], bf16, tag="ohS")
                nc.vector.tensor_tensor(
                    out=s_all[:], in0=esrc[:].broadcast_to([128, EC, 128]),
                    in1=iota8[:], op=ALU.is_equal)
                d_all = wpool.tile([128, EC, 128], bf16, tag="ohD")
                nc.vector.tensor_tensor(
                    out=d_all[:], in0=edst[:].broadcast_to([128, EC, 128]),
                    in1=iota8[:], op=ALU.is_equal)
                for c in range(EC):
                    nc.tensor.matmul(at_ps[:], s_all[:, c, :], d_all[:, c, :],
                                     start=(c == 0), stop=False)
                # += I (self-loops) via identity outer product, exact in bf16
                nc.tensor.matmul(at_ps[:], ident_b[:], ident_b[:], start=False,
                                 stop=True)
                ati = wpool.tile([128, 128], bf16, tag="ati", bufs=BL)
                nc.scalar.copy(ati[:], at_ps[:])
                atis.append(ati)
                # deg[d] = sum_s ATI[s,d]  -> dinv = 1/sqrt(deg)
                deg_ps = psB.tile([128, 1], f32, tag="mm")
                nc.tensor.matmul(deg_ps[:], ati[:], ones_b[:],
                                 start=True, stop=True)
                sdeg = spool.tile([128, 1], f32, tag="sv")
                nc.scalar.activation(sdeg[:], deg_ps[:], AFT.Sqrt)
                dinv = spool.tile([128, 1], f32, tag="dinv", bufs=BL)
                nc.vector.reciprocal(dinv[:], sdeg[:])
                dinvs.append(dinv)

            # ---------- phase 1: gate + pooling + GCN per graph ----------
            _b1b = [None]
            _b2b = [None]
            for g in range(BL):
                subv = subvs[g]
                ati = atis[g]
                dinv = dinvs[g]

                cnt_ps = psC.tile([128, SC], f32, tag="cnt")
                nf_ps = psA.tile([128, H], f32, tag="nf")
                p_all = wpool.tile([128, SC, 128], bf16, tag="ohP")
                nc.vector.tensor_tensor(
                    out=p_all[:], in0=subv[:].broadcast_to([128, SC, 128]),
                    in1=iota8[:, 0:SC, :], op=ALU.is_equal)
                for c in range(SC):
                    lht = lhpool.tile([128, H], f32r, tag="lh")
                    nc.sync.dma_start(lht[:], lh_d[g, c * 128 : (c + 1) * 128, :])
                    scr = scpool.tile([128, H], bf16, tag="scr")
                    logits = spool.tile([128, 1], f32, tag="sv")
                    nc.vector.scalar_tensor_tensor(
                        scr[:], lht[:].bitcast(f32), 0.0, wrb[:], ALU.bypass,
                        ALU.mult, accum_out=logits[:])
                    gate = spool.tile([128, 1], f32, tag="sv")
                    nc.scalar.activation(gate[:], logits[:], AFT.Sigmoid,
                                         bias=float(br_val))
                    pg_t = wpool.tile([128, 128], f32r, tag="ohPg")
                    nc.scalar.mul(pg_t[:], p_all[:, c, :], gate[:])
                    nc.tensor.matmul(cnt_ps[:, c : c + 1], p_all[:, c, :],
                                     ones_b[:], start=True, stop=True)
                    # pooled node feats: nf[n,h] += Pg[s,n]^T lh[s,h]
                    nc.tensor.matmul(nf_ps[:, 0:512], pg_t[:], lht[:, 0:512],
                                     start=(c == 0), stop=(c == SC - 1))
                    nc.tensor.matmul(nf_ps[:, 512:H], pg_t[:], lht[:, 512:H],
                                     start=(c == 0), stop=(c == SC - 1))

                # 1/max(cnt,1); combined layer-1 row scale s1 = invc * dinv
                cnt1 = spool.tile([128, 1], f32, tag="sv")
                nc.vector.tensor_reduce(cnt1[:], cnt_ps[:], mybir.AxisListType.X,
                                        ALU.add)
                mx = spool.tile([128, 1], f32, tag="sv")
                nc.vector.tensor_scalar_max(mx[:], cnt1[:], 1.0)
                invc = spool.tile([128, 1], f32, tag="sv")
                nc.vector.reciprocal(invc[:], mx[:])
                s1 = spool.tile([128, 1], f32, tag="sv")
                nc.vector.tensor_tensor(s1[:], invc[:], dinv[:], ALU.mult)

                # scale rows by s1 while moving PSUM->SBUF (bf16 for layer 1)
                nf_sb = wpool.tile([128, H], bf16, tag="nfsb", bufs=2)
                nc.vector.tensor_scalar_mul(nf_sb[:], nf_ps[:], s1[:])
                # transpose to nfT chunks [h,n]
                nfs = wpool.tile([128, HC, GH], bf16, tag="nfs", bufs=2)
                for hc in range(HC):
                    tr_ps = psB.tile([128, 128], bf16, tag="mm")
                    nc.tensor.transpose(tr_ps[:], nf_sb[:, hc * 128 : (hc + 1) * 128],
                                        ident_b[:])
                    nc.any.tensor_copy(nfs[:, hc, :], tr_ps[:])

                # GCN layer 1: T2 = (s1*sums) @ W1  (scale pre-applied)
                t1_ps = psB.tile([128, GH], f32, tag="mm")
                for hc in range(HC):
                    nc.tensor.matmul(t1_ps[:], nfs[:, hc, :], w1s[:, hc, :],
                                     start=(hc == 0), stop=(hc == HC - 1))
                t2 = wpool.tile([128, GH], bf16, tag="t2")
                nc.any.tensor_copy(t2[:], t1_ps[:])
                z_ps = psB.tile([128, GH], f32, tag="mm")
                nc.tensor.matmul(z_ps[:], ati[:], t2[:], start=True, stop=True)
                x1 = wpool.tile([128, GH], bf16, tag="x1")
                if b1_zero:
                    # x1 = dinv * relu(z)  (valid since dinv > 0)
                    nc.vector.tensor_scalar(x1[:], z_ps[:], 0.0, dinv[:],
                                            ALU.max, ALU.mult)
                else:
                    if g == 0 and _b1b[0] is None:
                        _b1b[0] = cpool.tile([128, GH], f32, name="b1bt")
                        nc.scalar.dma_start(_b1b[0][:], b1b_d[:])
                    x1p = wpool.tile([128, GH], f32, tag="x1p")
                    nc.vector.scalar_tensor_tensor(x1p[:], z_ps[:], dinv[:],
                                                   _b1b[0][:], ALU.mult, ALU.add)
                    nc.vector.tensor_scalar_max(x1[:], x1p[:], 0.0)

                # GCN layer 2
                x1t_ps = psB.tile([128, GH], bf16, tag="mm")
                nc.tensor.transpose(x1t_ps[:], x1[:], ident_b[:])
                x1t = wpool.tile([128, GH], bf16, tag="x1t")
                nc.any.tensor_copy(x1t[:], x1t_ps[:])
                tp_ps = psB.tile([128, GH], f32, tag="mm")
                nc.tensor.matmul(tp_ps[:], x1t[:], w2s[:], start=True, stop=True)
                t2p = wpool.tile([128, GH], bf16, tag="t2")
                nc.vector.tensor_scalar_mul(t2p[:], tp_ps[:], dinv[:])
                z2_ps = psB.tile([128, GH], f32, tag="mm")
                nc.tensor.matmul(z2_ps[:], ati[:], t2p[:], start=True, stop=True)
                x2 = wpool.tile([128, GH], bf16, tag="x1")
                if b2_zero:
                    nc.vector.tensor_scalar(x2[:], z2_ps[:], 0.0, dinv[:],
                                            ALU.max, ALU.mult)
                else:
                    if g == 0 and _b2b[0] is None:
                        _b2b[0] = cpool.tile([128, GH], f32, name="b2bt")
                        nc.scalar.dma_start(_b2b[0][:], b2b_d[:])
                    x2p = wpool.tile([128, GH], f32, tag="x1p")
                    nc.vector.scalar_tensor_tensor(x2p[:], z2_ps[:], dinv[:],
                                                   _b2b[0][:], ALU.mult, ALU.add)
                    nc.vector.tensor_scalar_max(x2[:], x2p[:], 0.0)

                # graph mean pool -> column g of catT6
                pool_ps = psB.tile([128, 1], f32, tag="mm")
                nc.tensor.matmul(pool_ps[:], x2[:], mean_b[:], start=True,
                                 stop=True)
                nc.scalar.copy(catT6[:, g : g + 1], pool_ps[:])

            # ---------- FC head over all BL graphs ----------
            ident_f = cpool.tile([128, 128], f32)
            nc.scalar.dma_start(ident_f[:], identf_d[:])
            wf1s = cpool.tile([128, FC, FH], f32)
            nc.scalar.dma_start(wf1s[:], wf1t_d[:])
            wf2s = cpool.tile([128, 2, L], f32)
            nc.scalar.dma_start(wf2s[:], wf2t_d[:])
            bf1b = cpool.tile([BL, FH], f32)
            nc.scalar.dma_start(bf1b[:], bf1b_d[:])
            bf2b = cpool.tile([BL, L], f32)
            nc.scalar.dma_start(bf2b[:], bf2b_d[:])
            clsr = cpool.tile([BL, H], f32)
            nc.sync.dma_start(clsr[:], lh_d[:, 0, :].bitcast(f32))
            h1_ps = psB.tile([BL, FH], f32, tag="mm")
            for c in range(FC):
                if c < HC:
                    ct_ps = psB.tile([128, BL], f32, tag="mm")
                    nc.tensor.transpose(ct_ps[:], clsr[:, c * 128 : (c + 1) * 128],
                                        ident_f[0:BL, 0:BL])
                    catc = wpool.tile([128, BL], f32, tag="catc", bufs=2)
                    nc.any.tensor_copy(catc[:], ct_ps[:])
                else:
                    catc = catT6
                nc.tensor.matmul(h1_ps[:], catc[:], wf1s[:, c, :], start=(c == 0),
                                 stop=(c == FC - 1))
            h1s = wpool.tile([BL, FH], f32, tag="h1")
            nc.vector.scalar_tensor_tensor(h1s[:], h1_ps[:], 1.0, bf1b[:],
                                           ALU.bypass, ALU.add)
            hr = wpool.tile([BL, FH], f32, tag="h1")
            nc.vector.tensor_scalar_max(hr[:], h1s[:], 0.0)
            out_ps = psB.tile([BL, L], f32, tag="mm")
            for c in range(2):
                ht_ps = psB.tile([128, BL], f32, tag="mm")
                nc.tensor.transpose(ht_ps[:], hr[:, c * 128 : (c + 1) * 128],
                                    ident_f[0:BL, 0:BL])
                htc = wpool.tile([128, BL], f32, tag="catc", bufs=2)
                nc.any.tensor_copy(htc[:], ht_ps[:])
                nc.tensor.matmul(out_ps[:], htc[:], wf2s[:, c, :], start=(c == 0),
                                 stop=(c == 1))
            outs = wpool.tile([BL, L], f32, tag="outs")
            nc.vector.scalar_tensor_tensor(outs[:], out_ps[:], 1.0, bf2b[:],
                                           ALU.bypass, ALU.add)
            nc.sync.dma_start(out_d[:], outs[:])

    _split_multi_waits(nc)
    return nc


def _prepare_in_maps(inputs):
    lh = np.ascontiguousarray(np.asarray(inputs["last_hidden"], dtype=np.float32))
    submap = np.asarray(inputs["submap"]).astype(np.int64)
    edge_index = np.asarray(inputs["edge_index"]).astype(np.int64)
    assert lh.shape == (B, S, H)
    assert int(inputs.get("num_nodes", N)) == N

    wr = np.asarray(inputs["wr"], dtype=np.float32)
    br = float(np.asarray(inputs["br"], dtype=np.float32))
    W1 = np.asarray(inputs["W1"], dtype=np.float32)
    b1 = np.asarray(inputs["b1"], dtype=np.float32)
    W2 = np.asarray(inputs["W2"], dtype=np.float32)
    b2 = np.asarray(inputs["b2"], dtype=np.float32)
    Wf1 = np.asarray(inputs["Wf1"], dtype=np.float32)
    bf1 = np.asarray(inputs["bf1"], dtype=np.float32)
    Wf2 = np.asarray(inputs["Wf2"], dtype=np.float32)
    bf2 = np.asarray(inputs["bf2"], dtype=np.float32)

    # Shared (replicated) tensors.
    consts = {
        "wrb": np.ascontiguousarray(np.broadcast_to(wr, (128, H))),
        "w1t": np.ascontiguousarray(
            W1.reshape(HC, 128, GH).transpose(1, 0, 2)).astype(BF16),
        "w2t": np.ascontiguousarray(W2).astype(BF16),
        "wf1t": np.ascontiguousarray(
            Wf1.reshape(FC, 128, FH).transpose(1, 0, 2)),
        "wf2t": np.ascontiguousarray(
            Wf2.reshape(2, 128, L).transpose(1, 0, 2)),
        "b1b": np.ascontiguousarray(np.broadcast_to(b1, (128, GH))),
        "b2b": np.ascontiguousarray(np.broadcast_to(b2, (128, GH))),
        "bf1b": np.ascontiguousarray(np.broadcast_to(bf1, (BL, FH))),
        "bf2b": np.ascontiguousarray(np.broadcast_to(bf2, (BL, L))),
        "iota_f": np.ascontiguousarray(
            np.broadcast_to(np.arange(128, dtype=np.float32), (128, 128))),
        "iota8": np.ascontiguousarray(
            np.broadcast_to(np.arange(128, dtype=np.float32), (128, EC, 128))),
        "ident_b": np.eye(128, dtype=np.float32).astype(BF16),
        "ident_f": np.eye(128, dtype=np.float32),
        "ones_r": np.ones((128, 1), np.float32),
        "ones_b": np.ones((128, 1), np.float32).astype(BF16),
        "mean_b": np.full((128, 1), 1.0 / N, np.float32).astype(BF16),
    }

    # Per-graph index layouts: value of token t goes to partition t%128,
    # column t//128.
    subv = submap.reshape(B, SC, 128).transpose(0, 2, 1).astype(np.float32)
    esrc = edge_index[:, 0, :].reshape(B, EC, 128).transpose(0, 2, 1).astype(np.float32)
    edst = edge_index[:, 1, :].reshape(B, EC, 128).transpose(0, 2, 1).astype(np.float32)

    in_maps = []
    for i in range(NCORES):
        sl = slice(i * BL, (i + 1) * BL)
        m = dict(consts)
        m["lh"] = np.ascontiguousarray(lh[sl])
        m["subv"] = np.ascontiguousarray(subv[sl])
        m["esrc"] = np.ascontiguousarray(esrc[sl])
        m["edst"] = np.ascontiguousarray(edst[sl])
        in_maps.append(m)
    flags = (br, bool(np.all(b1 == 0)), bool(np.all(b2 == 0)))
    return in_maps, flags


def _run(inputs, trace=False):
    in_maps, flags = _prepare_in_maps(inputs)
    key = ("prog",) + flags
    if key not in _CACHE:
        _CACHE[key] = build_program(*flags)
    nc = _CACHE[key]
    res = run_bass_kernel_spmd(nc, in_maps, list(range(NCORES)), trace=trace)
    out = np.concatenate([np.asarray(res.results[i]["out"]) for i in range(NCORES)],
                         axis=0).astype(np.float32)
    return out, res


def kernel(**inputs) -> np.ndarray:
    out, _ = _run(inputs, trace=False)
    return out
